# revision 24
# baseline (speedup 1.0000x reference)
"""Multi-head attention (B=4, L=2048, D=512, H=8) on 8 Trainium2 NeuronCores.

Sharding: core = (batch b, head-group hg) -> each core handles 1 batch and 4
heads (tensor-parallel column-shard of Wq/Wk/Wv, row-shard of Wo). The two
head-group partial outputs per batch are summed on the host (the TP
all-reduce step of the gather).

v2 engine plan (over the 136us baseline):
  - All DMA'd operands bf16; PE matmuls bf16 with f32 PSUM accumulation.
  - kh stored per-head zero-padded to 128 contraction rows (no PE tiling-mode
    switches; MM time is N-bound so the pad rows are free).
  - Input DMAs merged: kT / weight-blob / qT(2) / vT / mask = 6 issues
    (each DMA_DIRECT2D costs ~0.65us of serial Sync issue time).
  - Deep software pipeline in the attention loop: ctx matmuls run TWO
    iterations behind scores (ctx queue), and each iteration emits ctx
    BEFORE scores so the exp WAR on the single-buffered score PSUM clears
    before the next scores land. PSUM: s0[128,1024] + s1[128,1024] +
    2x ctx[65,1024] = 8 banks.
  - exp split: head0 exact ScalarE ACT (one [128,1024] op), head1 one-shot
    VectorE Schraudolph bf16-bitcast tensor_scalar (renormalization cancels
    the common-mode approximation error).
  - Normalize (deferred into the next phase, one piece per iteration):
      srow:  ScalarE copy ctxp[64:65] -> [1,1024]   (sums row, ones-col trick)
      drain: ScalarE/DVE copy ctxp[0:64] -> stage   (frees ctx psum early)
      recip: DVE reciprocal_approx_fast on srow
      bcast: GpSimd partition_broadcast -> bc[64,1024]
      mult:  head0 GpSimd tensor_tensor (all partition-base-0, aligned),
             head1 DVE tensor_tensor (partition-base shift needs DVE)
    At each phase end the ctx queue is drained completely (the last exps are
    just barely done by the time the PE reaches the popped ctx matmuls), so
    the freeing chain starts immediately at the phase boundary.
  - Output projection: q-half 0 interleaved one l-chunk per iteration late in
    phases (1,0)/(1,1) with all drains on ScalarE; q-half 1 as the tail.
    Output DMA'd bf16 (TP partials summed f32 on host).
  - Projection drains split ScalarE(hp0)/VectorE(hp1) so both engine FIFOs
    stay short ahead of the first exp.
  - Host-side key compaction (masked keys dropped) as in the baseline.
"""
import os
import sys
from collections import deque

import numpy as np

# a wedged NeuronCore (stuck engine state after a killed run) silently
# produces deterministic garbage; resetting cores at runtime init is cheap
os.environ.setdefault("NEURON_RT_RESET_CORES", "1")

for _p in ("/opt/trn_rl_repo", "/root/.axon_site/_ro/trn_rl_repo"):
    if os.path.isdir(_p) and _p not in sys.path:
        sys.path.insert(0, _p)

B, L, D, H = 4, 2048, 512, 8
DK = D // H          # 64
HPG = 4              # heads per group
GD = HPG * DK        # 256
HV = HPG * 65        # v-proj width (per-head mask col + 64 dims)
P = 128
NLB = L // 512       # 4 l-blocks of 512
NLC = L // P         # 16 l chunks

A16 = 128.0 / np.log(2.0)    # Schraudolph bf16 scale
B16 = 16247.9                # zero-mean bias (tuned in simulation)
NJUNK = 1                    # junk LDWEIGHTS per iteration (HAM heater)

_CACHE: dict = {}
_RUN_OPTS: dict = {"trace": False}


def _build_nc(ndc: int, nkc: int):
    """Build + compile the Bass program.

    ndc: 4 normally, 5 when q/k/v biases are nonzero (extra contraction chunk
    carrying a ones row x bias row).
    nkc: number of 128-key chunks after host-side compaction of masked keys.
    """
    from contextlib import ExitStack

    import concourse.bacc as bacc
    import concourse.tile as tile
    from concourse import mybir

    f32 = mybir.dt.float32
    bf16 = mybir.dt.bfloat16
    i16 = mybir.dt.int16
    EXP = mybir.ActivationFunctionType.Exp
    MULT = mybir.AluOpType.mult
    ADD = mybir.AluOpType.add

    nc = bacc.Bacc("TRN2", target_bir_lowering=False, debug=False, num_devices=8)

    NKP = nkc * P
    NKB = (NKP + 511) // 512
    # weight blob layout (free-dim element offsets)
    WK0 = 0
    WQ0 = WK0 + ndc * GD
    WV0 = WQ0 + ndc * GD
    WO0 = WV0 + ndc * HV
    WTOT = WO0 + 2 * D

    kT = nc.dram_tensor("kT", [P, ndc * NKP], bf16, kind="ExternalInput").ap()
    wbT = nc.dram_tensor("wbT", [P, WTOT], bf16, kind="ExternalInput").ap()
    qTa = nc.dram_tensor("qTa", [P, ndc * 1024], bf16, kind="ExternalInput").ap()
    qTb = nc.dram_tensor("qTb", [P, ndc * 1024], bf16, kind="ExternalInput").ap()
    vT = nc.dram_tensor("vT", [P, ndc * NKP], bf16, kind="ExternalInput").ap()
    maskT = nc.dram_tensor("maskT", [P, nkc], f32, kind="ExternalInput").ap()
    o = nc.dram_tensor("o", [NLC, P, D], bf16, kind="ExternalOutput").ap()

    with ExitStack() as ctx:
        tc = ctx.enter_context(tile.TileContext(nc))
        const = ctx.enter_context(tc.tile_pool(name="const", bufs=1))
        persist = ctx.enter_context(tc.tile_pool(name="persist", bufs=1))

        wb_sb = const.tile([P, WTOT], bf16)
        maskp_sb = const.tile([P, nkc], f32)
        dummy_sb = const.tile([1, 8], f32)
        junk = const.tile([P, 512], bf16)
        nc.vector.memset(junk, 0.0)
        # preload the exp table set early (overlaps the projection phase)
        nc.vector.memset(dummy_sb, 0.0)
        nc.scalar.activation(dummy_sb, dummy_sb, EXP)

        def wk_ap(dc):
            return wb_sb[:, WK0 + dc * GD:WK0 + (dc + 1) * GD]

        def wq_ap(dc):
            return wb_sb[:, WQ0 + dc * GD:WQ0 + (dc + 1) * GD]

        def wv_ap(dc):
            return wb_sb[:, WV0 + dc * HV:WV0 + (dc + 1) * HV]

        def wo_ap(c2):
            return wb_sb[:, WO0 + c2 * D:WO0 + (c2 + 1) * D]

        # persistent activations. kh per-head zero-padded to 128 rows.
        qh_sb = [persist.tile([P, L], bf16, name=f"qh{i}") for i in range(2)]
        khp_sb = [[persist.tile([P, NKP], bf16, name=f"khp{i}{j}")
                   for j in range(2)] for i in range(2)]
        # vh col DK(64) = mask/ones column (sums -> ctxp row 64; engine APs
        # must start at 32-aligned partitions, so the ctx rows stay at 0-63)
        vh_sb = persist.tile([P, nkc, HPG, 65], bf16, name="vh")
        ctx_sb = [persist.tile([P, L], bf16, name=f"ctx{i}") for i in range(2)]

        # ---------------- projections ----------------
        with tc.tile_pool(name="xT", bufs=1) as xpool, \
             tc.tile_pool(name="ppsum", bufs=6, space="PSUM") as ppsum:
            # HAM warm-up while the first input DMAs are in flight
            warm = ppsum.tile([P, 512], f32, tag="pp", name="warm")
            for _ in range(24):
                nc.tensor.matmul(warm[:, 0:256], lhsT=junk[:, 0:P],
                                 rhs=junk[:, 0:256], start=True, stop=True)
            for _ in range(20):
                nc.tensor.ldweights(junk[:, 0:P])
            kx = xpool.tile([P, ndc, NKP], bf16, tag="xk", name="kx")
            nc.sync.dma_start(kx, kT.rearrange("p (c w) -> p c w", c=ndc))
            nc.sync.dma_start(wb_sb, wbT)
            qxa = xpool.tile([P, ndc, 1024], bf16, tag="xqa", name="qxa")
            nc.sync.dma_start(qxa, qTa.rearrange("p (c w) -> p c w", c=ndc))
            qxb = xpool.tile([P, ndc, 1024], bf16, tag="xqb", name="qxb")
            nc.sync.dma_start(qxb, qTb.rearrange("p (c w) -> p c w", c=ndc))
            vx = xpool.tile([P, ndc, NKP], bf16, tag="xv", name="vx")
            nc.sync.dma_start(vx, vT.rearrange("p (c w) -> p c w", c=ndc))
            nc.sync.dma_start(maskp_sb, maskT)
            # prewarm the GpSimd custom-op library after input DMAs queued
            # (first partition_broadcast otherwise pays a ~6us IRAM load)
            dummy2_sb = const.tile([1, 8], f32)
            nc.gpsimd.partition_broadcast(dummy2_sb, dummy_sb)
            # khp zero-pad memsets early on DVE
            for hp in range(2):
                for hi in range(2):
                    nc.vector.memset(khp_sb[hp][hi], 0.0)
            # k projection -> khp (per-head zero-padded)
            kps = {}
            for dc in range(ndc):
                for hp in range(2):
                    for lb in range(NKB):
                        nb = min(512, NKP - lb * 512)
                        if dc == 0:
                            kps[hp, lb] = ppsum.tile([P, 512], f32, tag="pp",
                                                     name="ps_k")
                        nc.tensor.matmul(
                            kps[hp, lb][:, 0:nb],
                            lhsT=wk_ap(dc)[:, hp * P:(hp + 1) * P],
                            rhs=kx[:, dc, lb * 512:lb * 512 + nb],
                            start=(dc == 0),
                            stop=(dc == ndc - 1),
                        )
            # drains: hp0 on ScalarE, hp1 on VectorE (parallel FIFOs)
            for (hp, lb), ps in kps.items():
                nb = min(512, NKP - lb * 512)
                for hi in range(2):
                    hb = hi * DK
                    dst = khp_sb[hp][hi][hb:hb + DK, lb * 512:lb * 512 + nb]
                    if hp == 0:
                        nc.scalar.copy(dst, ps[hb:hb + DK, 0:nb])
                    else:
                        nc.vector.tensor_copy(dst, ps[hb:hb + DK, 0:nb])
            # q projection, hp0 first (gates attention start)
            for hp in range(2):
                for lb in range(NLB):
                    qx = qxa if lb < 2 else qxb
                    col = (lb % 2) * 512
                    ps = ppsum.tile([P, 512], f32, tag="pp", name="ps_q")
                    for dc in range(ndc):
                        nc.tensor.matmul(
                            ps,
                            lhsT=wq_ap(dc)[:, hp * P:(hp + 1) * P],
                            rhs=qx[:, dc, col:col + 512],
                            start=(dc == 0),
                            stop=(dc == ndc - 1),
                        )
                    dst = qh_sb[hp][:, lb * 512:(lb + 1) * 512]
                    if hp == 0:
                        nc.scalar.copy(dst, ps)
                    else:
                        nc.vector.tensor_copy(dst, ps)
            # v projection: vh[l, :] with mask fold (keys on partitions)
            for lc in range(nkc):
                ps = ppsum.tile([P, 512], f32, tag="pp", name="ps_v")[:, 0:HV]
                for dc in range(ndc):
                    nc.tensor.matmul(
                        ps,
                        lhsT=vx[:, dc, lc * P:(lc + 1) * P],
                        rhs=wv_ap(dc),
                        start=(dc == 0),
                        stop=(dc == ndc - 1),
                    )
                nc.vector.tensor_scalar_mul(
                    vh_sb[:, lc, :, :], ps.rearrange("p (h d) -> p h d", h=HPG),
                    maskp_sb[:, lc:lc + 1],
                )
                # mask column -> 0/1 (weights there are zero)
                nc.vector.tensor_copy(
                    vh_sb[:, lc, :, DK:DK + 1],
                    maskp_sb[:, lc:lc + 1, None].to_broadcast((P, HPG, 1)),
                )

        # ---------------- attention ----------------
        with tc.tile_pool(name="spsum", bufs=1, space="PSUM") as s_pool, \
             tc.tile_pool(name="cpsum", bufs=2, space="PSUM") as ctx_pool, \
             tc.tile_pool(name="pt", bufs=4) as pt_pool, \
             tc.tile_pool(name="nrm", bufs=4) as nrm_pool, \
             tc.tile_pool(name="osb", bufs=4) as o_pool:

            ot_state = {}
            S_TAGS = ("s0a", "s0b", "s1a", "s1b")

            def emit_oproj_lc(lc, on_scalar=True):
                # one l-chunk of the output projection (borrows s psum)
                gi = lc % 2
                if gi == 0:
                    ot_state["t"] = o_pool.tile([P, 2, D], bf16, tag="o",
                                                name="ot")
                ot = ot_state["t"]
                ps = s_pool.tile([P, 512], f32, tag=S_TAGS[lc % 4], bufs=1,
                                 name="ps_o")
                for c2 in range(2):
                    nc.tensor.matmul(
                        ps,
                        lhsT=ctx_sb[c2][:, lc * P:(lc + 1) * P],
                        rhs=wo_ap(c2),
                        start=(c2 == 0), stop=(c2 == 1),
                    )
                if on_scalar:
                    nc.scalar.copy(ot[:, gi, :], ps)
                else:
                    nc.vector.tensor_copy(ot[:, gi, :], ps)
                if gi == 1:
                    nc.sync.dma_start(
                        o[lc - 1:lc + 1].rearrange("l p d -> p l d"), ot)

            def emit_ctx(ent, hi):
                # ctx accumulation for one delayed iteration, one head
                ctxp_e, hp_e, pts, kcp = ent
                pt = pts[hi]
                vlhsT = vh_sb[:, kcp, 2 * hp_e + hi, :]
                for j in range(2):
                    nc.tensor.matmul(
                        ctxp_e[hi][:, j * 512:(j + 1) * 512],
                        lhsT=vlhsT,
                        rhs=pt[:, j * 512:(j + 1) * 512],
                        start=(kcp == 0), stop=(kcp == nkc - 1),
                    )

            # normalize chain pieces (phase-carried)
            def n_srow(st, hi):
                st["srow"][hi] = nrm_pool.tile([1, 1024], f32, tag="srow",
                                               name="srow")
                nc.scalar.copy(st["srow"][hi], st["ctxp"][hi][64:65, :])

            def n_drain(st, hi, on_scalar):
                st["stage"][hi] = nrm_pool.tile([DK, 1024], f32, tag="stage",
                                                name="stage")
                if on_scalar:
                    nc.scalar.copy(st["stage"][hi], st["ctxp"][hi][0:DK, :])
                else:
                    nc.vector.tensor_copy(st["stage"][hi],
                                          st["ctxp"][hi][0:DK, :])

            def n_recip(st, hi):
                st["rrow"][hi] = nrm_pool.tile([1, 1024], f32, tag="rrow",
                                               name="rrow")
                nc.vector.reciprocal_approx_fast(st["rrow"][hi],
                                                 st["srow"][hi])

            def n_bcast(st, hi):
                st["bc"][hi] = nrm_pool.tile([DK, 1024], f32, tag="bc",
                                             name="bc")
                nc.gpsimd.partition_broadcast(st["bc"][hi], st["rrow"][hi])

            def n_mult(st, hi):
                # GpSimd is ~6x slower than DVE for 2-input elementwise
                # (each Q7 core serializes its 16 partitions): DVE only.
                hb = hi * DK
                nc.vector.tensor_tensor(
                    st["ctx_dst"][hb:hb + DK, st["q0"]:st["q0"] + 1024],
                    st["stage"][hi], st["bc"][hi], MULT)

            PHASES = [(0, 0), (0, 1), (1, 0), (1, 1)]
            ctxq = deque()
            pend = None      # normalize state of the previous phase
            for ph, (q2, hp) in enumerate(PHASES):
                q0 = q2 * 1024
                ctxp = [ctx_pool.tile([65, 1024], f32, tag="ctx",
                                      name=f"ctx{hi}") for hi in range(2)]
                # per-iteration extras: {kc: [callable, ...]}
                extras = {}

                def _at(kc_t, fn, _e=extras):
                    kc_t = max(0, min(kc_t, nkc - 1))
                    _e.setdefault(kc_t, []).append(fn)

                if pend is not None:
                    # srow0/drain0 were emitted at the previous phase's end
                    pp = pend
                    _at(0, lambda st=pp: n_srow(st, 1))
                    _at(1, lambda st=pp: n_drain(st, 1, True))
                    _at(1, lambda st=pp: n_recip(st, 0))
                    _at(2, lambda st=pp: n_recip(st, 1))
                    _at(2, lambda st=pp: n_bcast(st, 0))
                    _at(3, lambda st=pp: n_bcast(st, 1))
                    _at(4, lambda st=pp: n_mult(st, 0))
                    _at(5, lambda st=pp: n_mult(st, 1))
                if ph == 2:
                    for lc in range(3):         # lc 0..2 at kc nkc-3..nkc-1
                        _at(nkc - 3 + lc, lambda lc_=lc: emit_oproj_lc(
                            lc_, on_scalar=(lc_ % 2 == 0)))
                if ph == 3:
                    for lc in range(3, 8):      # lc 3..7 at kc 2..6
                        _at(lc - 1, lambda lc_=lc: emit_oproj_lc(
                            lc_, on_scalar=(lc_ % 2 == 0)))

                for kc in range(nkc):
                    if kc == 0 and ph > 0:
                        # phase-boundary HAM heater
                        for _ in range(2):
                            nc.tensor.ldweights(junk[:, 0:P])
                    # delayed ctx: 2 iterations behind scores
                    if ctxq and kc >= 2:
                        ent = ctxq.popleft()
                        emit_ctx(ent, 0)
                        emit_ctx(ent, 1)
                    # scores: each half of each head's [128,1024] is its own
                    # single-bank psum tile with its own exp op, so the
                    # scores->exp->scores WAR loop runs per-half and leaves
                    # ~1us/iter of slack on Sc/DVE to absorb the extras
                    s0h = [s_pool.tile([P, 512], f32, tag=S_TAGS[j], bufs=1,
                                       name=S_TAGS[j]) for j in range(2)]
                    s1h = [s_pool.tile([P, 512], f32, tag=S_TAGS[2 + j],
                                       bufs=1, name=S_TAGS[2 + j])
                           for j in range(2)]
                    for j in range(2):
                        nc.tensor.matmul(
                            s0h[j],
                            lhsT=khp_sb[hp][0][:, kc * P:(kc + 1) * P],
                            rhs=qh_sb[hp][:, q0 + j * 512:q0 + (j + 1) * 512],
                            start=True, stop=True,
                        )
                    for j in range(2):
                        nc.tensor.matmul(
                            s1h[j],
                            lhsT=khp_sb[hp][1][:, kc * P:(kc + 1) * P],
                            rhs=qh_sb[hp][:, q0 + j * 512:q0 + (j + 1) * 512],
                            start=True, stop=True,
                        )
                    for _ in range(NJUNK):
                        nc.tensor.ldweights(junk[:, 0:P])
                    # exp: head0 exact ScalarE, head1 Schraudolph VectorE.
                    # On normalize-carrying phases two h1 exps move to
                    # ScalarE to rebalance (DVE carries recip+mult there).
                    pt0 = pt_pool.tile([P, 1024], bf16, tag="pt0", name="pt0")
                    for j in range(2):
                        nc.scalar.activation(pt0[:, j * 512:(j + 1) * 512],
                                             s0h[j], EXP)
                    if pend is not None and kc == 6:
                        pt1b = pt_pool.tile([P, 1024], bf16, tag="pt1x",
                                            name="pt1x")
                        for j in range(2):
                            nc.scalar.activation(
                                pt1b[:, j * 512:(j + 1) * 512], s1h[j], EXP)
                        pt1 = pt1b
                    else:
                        pt1i = pt_pool.tile([P, 1024], i16, tag="pt1",
                                            name="pt1")
                        for j in range(2):
                            nc.vector.tensor_scalar(
                                pt1i[:, j * 512:(j + 1) * 512], s1h[j],
                                float(A16), float(B16), MULT, ADD)
                        pt1 = pt1i.bitcast(bf16)
                    ctxq.append((ctxp, hp, (pt0, pt1), kc))
                    for fn in extras.get(kc, ()):
                        fn()
                    if kc == nkc - 1:
                        # phase end: drain the queue completely (the last
                        # exps are just done by the time PE reaches these)
                        while ctxq:
                            ent = ctxq.popleft()
                            emit_ctx(ent, 0)
                            emit_ctx(ent, 1)
                pend = {"ctxp": ctxp, "ctx_dst": ctx_sb[hp], "q0": q0,
                        "srow": {}, "stage": {}, "rrow": {}, "bc": {}}
                # slot-freeing pieces right at the phase end: both engines
                # have an idle window here (the pop-all iteration is PE-long)
                n_srow(pend, 0)
                n_drain(pend, 0, False)

            # ---------------- tail ----------------
            # pre-issue the c2=0 half of oproj lc8-11 (ctx_sb[0] is ready)
            # so the PE stays busy+warm through the final normalize chain
            def oproj_start(lc):
                gi = lc % 2
                if gi == 0:
                    ot_state["t"] = o_pool.tile([P, 2, D], bf16, tag="o",
                                                name="ot")
                ps = s_pool.tile([P, 512], f32, tag=S_TAGS[lc % 4], bufs=1,
                                 name="ps_o")
                nc.tensor.matmul(ps, lhsT=ctx_sb[0][:, lc * P:(lc + 1) * P],
                                 rhs=wo_ap(0), start=True, stop=False)
                return ps, ot_state["t"]

            def oproj_finish(lc, ps, ot, on_scalar):
                nc.tensor.matmul(ps, lhsT=ctx_sb[1][:, lc * P:(lc + 1) * P],
                                 rhs=wo_ap(1), start=False, stop=True)
                gi = lc % 2
                if on_scalar:
                    nc.scalar.copy(ot[:, gi, :], ps)
                else:
                    nc.vector.tensor_copy(ot[:, gi, :], ps)
                if gi == 1:
                    nc.sync.dma_start(
                        o[lc - 1:lc + 1].rearrange("l p d -> p l d"), ot)

            n_recip(pend, 0)
            parts = {}
            for lc in range(8, 12):
                parts[lc] = oproj_start(lc)
            n_srow(pend, 1)
            n_bcast(pend, 0)
            n_drain(pend, 1, True)
            n_recip(pend, 1)
            for _ in range(30):     # HAM heater through the chain wait
                nc.tensor.ldweights(junk[:, 0:P])
            n_mult(pend, 0)
            n_bcast(pend, 1)
            n_mult(pend, 1)
            for lc in range(8, 12):
                ps, ot = parts[lc]
                oproj_finish(lc, ps, ot, on_scalar=(lc % 2 == 0))
            for lc in range(12, 16):
                emit_oproj_lc(lc, on_scalar=(lc % 2 == 0))

    nc.compile()
    return nc


def _get_nc(ndc: int, nkc: int):
    key = ("nc", ndc, nkc)
    if key not in _CACHE:
        _CACHE[key] = _build_nc(ndc, nkc)
    return _CACHE[key]


def _prep_core(core, q, k, v, masks, wq_w, wq_b, wk_w, wk_b, wv_w, wv_b, ndc,
               nkc):
    import ml_dtypes

    bf16 = ml_dtypes.bfloat16
    b, hg = core // 2, core % 2
    rows = slice(hg * GD, (hg + 1) * GD)
    scale = np.float32(1.0 / np.sqrt(DK))
    NKP = nkc * P
    idx = np.nonzero(masks[b])[0]          # unmasked key positions

    def xt_flat(x, compact):
        w = NKP if compact else L
        xt = np.zeros((ndc, P, w), np.float32)
        xs = x[idx] if compact else x      # [nk or L, 512]
        xt[:4, :, :xs.shape[0]] = np.ascontiguousarray(xs.T).reshape(4, P, -1)
        if ndc == 5:
            xt[4, 0, :xs.shape[0]] = 1.0   # ones row for the bias chunk
        # [P, ndc, w] partition-major
        return np.ascontiguousarray(xt.transpose(1, 0, 2)).astype(bf16)

    def w_flat(wT, bias, width):
        w = np.zeros((ndc * P, width), np.float32)
        w[:D] = wT
        if ndc == 5:
            w[D] = bias
        return np.ascontiguousarray(
            w.reshape(ndc, P, width).transpose(1, 0, 2).reshape(P, -1))

    wqT = (wq_w[rows, :].T * scale).astype(np.float32)          # [512, 256]
    wkT = wk_w[rows, :].T.astype(np.float32)
    # v weights: mask/ones column LAST per head (col DK; sums -> ctxp row 64)
    wvT = np.zeros((D, HV), np.float32)
    wvb = np.zeros((HV,), np.float32)
    wvg = wv_w[rows, :]
    for hh in range(HPG):
        wvT[:, hh * 65:hh * 65 + DK] = wvg[hh * DK:(hh + 1) * DK].T
        wvb[hh * 65:hh * 65 + DK] = wv_b[rows][hh * DK:(hh + 1) * DK]
    maskc = np.zeros((NKP,), np.float32)
    maskc[:len(idx)] = 1.0
    qt = xt_flat(q[b], False)              # [P, ndc, L]
    return {
        "qTa": np.ascontiguousarray(qt[:, :, 0:1024].reshape(P, -1)),
        "qTb": np.ascontiguousarray(qt[:, :, 1024:2048].reshape(P, -1)),
        "kT": np.ascontiguousarray(xt_flat(k[b], True).reshape(P, -1)),
        "vT": np.ascontiguousarray(xt_flat(v[b], True).reshape(P, -1)),
        "_wk": w_flat(wkT, wk_b[rows], GD),
        "_wq": w_flat(wqT, wq_b[rows] * scale, GD),
        "_wv": w_flat(wvT, wvb, HV),
        "maskT": np.ascontiguousarray(
            maskc.reshape(nkc, P).T.astype(np.float32)),
    }


def kernel(q, k, v, masks, wq_w, wq_b, wk_w, wk_b, wv_w, wv_b, wo_w, wo_b):
    import ml_dtypes

    from concourse.bass_utils import run_bass_kernel_spmd

    bf16 = ml_dtypes.bfloat16
    q = np.asarray(q, np.float32)
    k = np.asarray(k, np.float32)
    v = np.asarray(v, np.float32)
    masks_np = np.asarray(masks)
    args = [np.asarray(a, np.float32) for a in
            (wq_w, wq_b, wk_w, wk_b, wv_w, wv_b, wo_w, wo_b)]
    wq_w, wq_b, wk_w, wk_b, wv_w, wv_b, wo_w, wo_b = args

    ndc = 5 if (np.any(wq_b) or np.any(wk_b) or np.any(wv_b)) else 4
    # key compaction: pad the max unmasked-key count to a 128 multiple
    max_nk = max(int(np.count_nonzero(masks_np[b])) for b in range(B))
    nkc = max(7, (max_nk + P - 1) // P)
    nc = _get_nc(ndc, nkc)

    in_maps = []
    for core in range(8):
        m = _prep_core(core, q, k, v, masks_np, wq_w, wq_b, wk_w, wk_b,
                       wv_w, wv_b, ndc, nkc)
        hg = core % 2
        rows = slice(hg * GD, (hg + 1) * GD)
        woT = np.ascontiguousarray(
            wo_w[:, rows].T.reshape(2, P, D).transpose(1, 0, 2).reshape(P, -1))
        m["wbT"] = np.concatenate(
            [m.pop("_wk"), m.pop("_wq"), m.pop("_wv"), woT],
            axis=1).astype(bf16)
        in_maps.append(m)

    res = run_bass_kernel_spmd(nc, in_maps, core_ids=list(range(8)),
                               trace=_RUN_OPTS.get("trace", False),
                               tmpdir=_RUN_OPTS.get("tmpdir"))
    _CACHE["last_result"] = res
    outs = res.results

    O = np.zeros((B, L, D), np.float32)
    for b in range(B):
        O[b] = (outs[2 * b]["o"].reshape(L, D).astype(np.float32)
                + outs[2 * b + 1]["o"].reshape(L, D).astype(np.float32))
    O += (wv_b @ wo_w.T + wo_b)[None, None, :] if ndc == 4 else wo_b[None, None, :]
    return O


# revision 25
# speedup vs baseline: 1.1736x; 1.1736x over previous
"""Multi-head attention (B=4, L=2048, D=512, H=8) on 8 Trainium2 NeuronCores.

Sharding: core = (batch b, head-group hg) -> each core handles 1 batch and 4
heads (tensor-parallel column-shard of Wq/Wk/Wv, row-shard of Wo). The two
head-group partial outputs per batch are summed on the host (the TP
all-reduce step of the gather).

v2 engine plan (over the 136us baseline):
  - All DMA'd operands bf16; PE matmuls bf16 with f32 PSUM accumulation.
  - kh stored per-head zero-padded to 128 contraction rows (no PE tiling-mode
    switches; MM time is N-bound so the pad rows are free).
  - Input DMAs merged: kT / weight-blob / qT(2) / vT / mask = 6 issues
    (each DMA_DIRECT2D costs ~0.65us of serial Sync issue time).
  - Deep software pipeline in the attention loop: ctx matmuls run TWO
    iterations behind scores (ctx queue), and each iteration emits ctx
    BEFORE scores so the exp WAR on the single-buffered score PSUM clears
    before the next scores land. PSUM: s0[128,1024] + s1[128,1024] +
    2x ctx[65,1024] = 8 banks.
  - exp split: head0 exact ScalarE ACT (one [128,1024] op), head1 one-shot
    VectorE Schraudolph bf16-bitcast tensor_scalar (renormalization cancels
    the common-mode approximation error).
  - Normalize (deferred into the next phase, one piece per iteration):
      srow:  ScalarE copy ctxp[64:65] -> [1,1024]   (sums row, ones-col trick)
      drain: ScalarE/DVE copy ctxp[0:64] -> stage   (frees ctx psum early)
      recip: DVE reciprocal_approx_fast on srow
      bcast: GpSimd partition_broadcast -> bc[64,1024]
      mult:  head0 GpSimd tensor_tensor (all partition-base-0, aligned),
             head1 DVE tensor_tensor (partition-base shift needs DVE)
    At each phase end the ctx queue is drained completely (the last exps are
    just barely done by the time the PE reaches the popped ctx matmuls), so
    the freeing chain starts immediately at the phase boundary.
  - Output projection: q-half 0 interleaved one l-chunk per iteration late in
    phases (1,0)/(1,1) with all drains on ScalarE; q-half 1 as the tail.
    Output DMA'd bf16 (TP partials summed f32 on host).
  - Projection drains split ScalarE(hp0)/VectorE(hp1) so both engine FIFOs
    stay short ahead of the first exp.
  - Host-side key compaction (masked keys dropped) as in the baseline.
"""
import os
import sys
from collections import deque

import numpy as np

# a wedged NeuronCore (stuck engine state after a killed run) silently
# produces deterministic garbage; resetting cores at runtime init is cheap
os.environ.setdefault("NEURON_RT_RESET_CORES", "1")

for _p in ("/opt/trn_rl_repo", "/root/.axon_site/_ro/trn_rl_repo"):
    if os.path.isdir(_p) and _p not in sys.path:
        sys.path.insert(0, _p)

B, L, D, H = 4, 2048, 512, 8
DK = D // H          # 64
HPG = 4              # heads per group
GD = HPG * DK        # 256
HV = HPG * 65        # v-proj width (per-head mask col + 64 dims)
P = 128
NLB = L // 512       # 4 l-blocks of 512
NLC = L // P         # 16 l chunks

A16 = 128.0 / np.log(2.0)    # Schraudolph bf16 scale
B16 = 16247.9                # zero-mean bias (tuned in simulation)
NJUNK = 1                    # junk LDWEIGHTS per iteration (HAM heater)

_CACHE: dict = {}
_RUN_OPTS: dict = {"trace": False}


def _build_nc(ndc: int, nkc: int):
    """Build + compile the Bass program.

    ndc: 4 normally, 5 when q/k/v biases are nonzero (extra contraction chunk
    carrying a ones row x bias row).
    nkc: number of 128-key chunks after host-side compaction of masked keys.
    """
    from contextlib import ExitStack

    import concourse.bacc as bacc
    import concourse.tile as tile
    from concourse import mybir

    f32 = mybir.dt.float32
    bf16 = mybir.dt.bfloat16
    i16 = mybir.dt.int16
    EXP = mybir.ActivationFunctionType.Exp
    MULT = mybir.AluOpType.mult
    ADD = mybir.AluOpType.add

    nc = bacc.Bacc("TRN2", target_bir_lowering=False, debug=False, num_devices=8)

    NKP = nkc * P
    NKB = (NKP + 511) // 512
    # weight blob layout (free-dim element offsets)
    WK0 = 0
    WQ0 = WK0 + ndc * GD
    WV0 = WQ0 + ndc * GD
    WO0 = WV0 + ndc * HV
    WTOT = WO0 + 2 * D

    kT = nc.dram_tensor("kT", [P, ndc * NKP], bf16, kind="ExternalInput").ap()
    wbT = nc.dram_tensor("wbT", [P, WTOT], bf16, kind="ExternalInput").ap()
    qTa = nc.dram_tensor("qTa", [P, ndc * 1024], bf16, kind="ExternalInput").ap()
    qTb = nc.dram_tensor("qTb", [P, ndc * 1024], bf16, kind="ExternalInput").ap()
    vT = nc.dram_tensor("vT", [P, ndc * NKP], bf16, kind="ExternalInput").ap()
    maskT = nc.dram_tensor("maskT", [P, nkc], f32, kind="ExternalInput").ap()
    o = nc.dram_tensor("o", [NLC, P, D], bf16, kind="ExternalOutput").ap()

    with ExitStack() as ctx:
        tc = ctx.enter_context(tile.TileContext(nc))
        const = ctx.enter_context(tc.tile_pool(name="const", bufs=1))
        persist = ctx.enter_context(tc.tile_pool(name="persist", bufs=1))

        wb_sb = const.tile([P, WTOT], bf16)
        maskp_sb = const.tile([P, nkc], f32)
        dummy_sb = const.tile([1, 8], f32)
        junk = const.tile([P, 512], bf16)
        nc.vector.memset(junk, 0.0)
        # preload the exp table set early (overlaps the projection phase)
        nc.vector.memset(dummy_sb, 0.0)
        nc.scalar.activation(dummy_sb, dummy_sb, EXP)

        def wk_ap(dc):
            return wb_sb[:, WK0 + dc * GD:WK0 + (dc + 1) * GD]

        def wq_ap(dc):
            return wb_sb[:, WQ0 + dc * GD:WQ0 + (dc + 1) * GD]

        def wv_ap(dc):
            return wb_sb[:, WV0 + dc * HV:WV0 + (dc + 1) * HV]

        def wo_ap(c2):
            return wb_sb[:, WO0 + c2 * D:WO0 + (c2 + 1) * D]

        # persistent activations. kh per-head zero-padded to 128 rows.
        qh_sb = [persist.tile([P, L], bf16, name=f"qh{i}") for i in range(2)]
        khp_sb = [[persist.tile([P, NKP], bf16, name=f"khp{i}{j}")
                   for j in range(2)] for i in range(2)]
        # vh col DK(64) = mask/ones column (sums -> ctxp row 64; engine APs
        # must start at 32-aligned partitions, so the ctx rows stay at 0-63)
        vh_sb = persist.tile([P, nkc, HPG, 65], bf16, name="vh")
        ctx_sb = [persist.tile([P, L], bf16, name=f"ctx{i}") for i in range(2)]

        # ---------------- projections ----------------
        with tc.tile_pool(name="xT", bufs=1) as xpool, \
             tc.tile_pool(name="ppsum", bufs=6, space="PSUM") as ppsum:
            # HAM warm-up while the first input DMAs are in flight
            warm = ppsum.tile([P, 512], f32, tag="pp", name="warm")
            for _ in range(24):
                nc.tensor.matmul(warm[:, 0:256], lhsT=junk[:, 0:P],
                                 rhs=junk[:, 0:256], start=True, stop=True)
            for _ in range(20):
                nc.tensor.ldweights(junk[:, 0:P])
            kx = xpool.tile([P, ndc, NKP], bf16, tag="xk", name="kx")
            nc.sync.dma_start(kx, kT.rearrange("p (c w) -> p c w", c=ndc))
            nc.sync.dma_start(wb_sb, wbT)
            qxa = xpool.tile([P, ndc, 1024], bf16, tag="xqa", name="qxa")
            nc.sync.dma_start(qxa, qTa.rearrange("p (c w) -> p c w", c=ndc))
            qxb = xpool.tile([P, ndc, 1024], bf16, tag="xqb", name="qxb")
            nc.sync.dma_start(qxb, qTb.rearrange("p (c w) -> p c w", c=ndc))
            vx = xpool.tile([P, ndc, NKP], bf16, tag="xv", name="vx")
            nc.sync.dma_start(vx, vT.rearrange("p (c w) -> p c w", c=ndc))
            nc.sync.dma_start(maskp_sb, maskT)
            # prewarm the GpSimd custom-op library after input DMAs queued
            # (first partition_broadcast otherwise pays a ~6us IRAM load)
            dummy2_sb = const.tile([1, 8], f32)
            nc.gpsimd.partition_broadcast(dummy2_sb, dummy_sb)
            # khp zero-pad memsets early on DVE
            for hp in range(2):
                for hi in range(2):
                    nc.vector.memset(khp_sb[hp][hi], 0.0)
            # k projection -> khp (per-head zero-padded)
            kps = {}
            for dc in range(ndc):
                for hp in range(2):
                    for lb in range(NKB):
                        nb = min(512, NKP - lb * 512)
                        if dc == 0:
                            kps[hp, lb] = ppsum.tile([P, 512], f32, tag="pp",
                                                     name="ps_k")
                        nc.tensor.matmul(
                            kps[hp, lb][:, 0:nb],
                            lhsT=wk_ap(dc)[:, hp * P:(hp + 1) * P],
                            rhs=kx[:, dc, lb * 512:lb * 512 + nb],
                            start=(dc == 0),
                            stop=(dc == ndc - 1),
                        )
            # drains: hp0 on ScalarE, hp1 on VectorE (parallel FIFOs)
            for (hp, lb), ps in kps.items():
                nb = min(512, NKP - lb * 512)
                for hi in range(2):
                    hb = hi * DK
                    dst = khp_sb[hp][hi][hb:hb + DK, lb * 512:lb * 512 + nb]
                    if hp == 0:
                        nc.scalar.copy(dst, ps[hb:hb + DK, 0:nb])
                    else:
                        nc.vector.tensor_copy(dst, ps[hb:hb + DK, 0:nb])
            # q projection, hp0 first (gates attention start)
            for hp in range(2):
                for lb in range(NLB):
                    qx = qxa if lb < 2 else qxb
                    col = (lb % 2) * 512
                    ps = ppsum.tile([P, 512], f32, tag="pp", name="ps_q")
                    for dc in range(ndc):
                        nc.tensor.matmul(
                            ps,
                            lhsT=wq_ap(dc)[:, hp * P:(hp + 1) * P],
                            rhs=qx[:, dc, col:col + 512],
                            start=(dc == 0),
                            stop=(dc == ndc - 1),
                        )
                    dst = qh_sb[hp][:, lb * 512:(lb + 1) * 512]
                    if hp == 0:
                        nc.scalar.copy(dst, ps)
                    else:
                        nc.vector.tensor_copy(dst, ps)
            # v projection: vh[l, :] with mask fold (keys on partitions)
            for lc in range(nkc):
                ps = ppsum.tile([P, 512], f32, tag="pp", name="ps_v")[:, 0:HV]
                for dc in range(ndc):
                    nc.tensor.matmul(
                        ps,
                        lhsT=vx[:, dc, lc * P:(lc + 1) * P],
                        rhs=wv_ap(dc),
                        start=(dc == 0),
                        stop=(dc == ndc - 1),
                    )
                nc.vector.tensor_scalar_mul(
                    vh_sb[:, lc, :, :], ps.rearrange("p (h d) -> p h d", h=HPG),
                    maskp_sb[:, lc:lc + 1],
                )
                # mask column -> 0/1 (weights there are zero)
                nc.vector.tensor_copy(
                    vh_sb[:, lc, :, DK:DK + 1],
                    maskp_sb[:, lc:lc + 1, None].to_broadcast((P, HPG, 1)),
                )

        # ---------------- attention ----------------
        with tc.tile_pool(name="spsum", bufs=1, space="PSUM") as s_pool, \
             tc.tile_pool(name="cpsum", bufs=2, space="PSUM") as ctx_pool, \
             tc.tile_pool(name="pt", bufs=4) as pt_pool, \
             tc.tile_pool(name="nrm", bufs=4) as nrm_pool, \
             tc.tile_pool(name="osb", bufs=4) as o_pool:

            ot_state = {}
            S_TAGS = ("s0a", "s0b", "s1a", "s1b")

            def emit_oproj_lc(lc, on_scalar=True):
                # one l-chunk of the output projection (borrows s psum)
                gi = lc % 2
                if gi == 0:
                    ot_state["t"] = o_pool.tile([P, 2, D], bf16, tag="o",
                                                name="ot")
                ot = ot_state["t"]
                ps = s_pool.tile([P, 512], f32, tag=S_TAGS[lc % 4], bufs=1,
                                 name="ps_o")
                for c2 in range(2):
                    nc.tensor.matmul(
                        ps,
                        lhsT=ctx_sb[c2][:, lc * P:(lc + 1) * P],
                        rhs=wo_ap(c2),
                        start=(c2 == 0), stop=(c2 == 1),
                    )
                if on_scalar:
                    nc.scalar.copy(ot[:, gi, :], ps)
                else:
                    nc.vector.tensor_copy(ot[:, gi, :], ps)
                if gi == 1:
                    nc.sync.dma_start(
                        o[lc - 1:lc + 1].rearrange("l p d -> p l d"), ot)

            def emit_ctx(ent, hi):
                # ctx accumulation for one delayed iteration, one head
                ctxp_e, hp_e, pts, kcp = ent
                pt = pts[hi]
                vlhsT = vh_sb[:, kcp, 2 * hp_e + hi, :]
                for j in range(2):
                    nc.tensor.matmul(
                        ctxp_e[hi][:, j * 512:(j + 1) * 512],
                        lhsT=vlhsT,
                        rhs=pt[:, j * 512:(j + 1) * 512],
                        start=(kcp == 0), stop=(kcp == nkc - 1),
                    )

            # normalize chain pieces (phase-carried)
            def n_srow(st, hi):
                st["srow"][hi] = nrm_pool.tile([1, 1024], f32, tag="srow",
                                               name="srow")
                nc.scalar.copy(st["srow"][hi], st["ctxp"][hi][64:65, :])

            def n_drain(st, hi, on_scalar):
                st["stage"][hi] = nrm_pool.tile([DK, 1024], f32, tag="stage",
                                                name="stage")
                if on_scalar:
                    nc.scalar.copy(st["stage"][hi], st["ctxp"][hi][0:DK, :])
                else:
                    nc.vector.tensor_copy(st["stage"][hi],
                                          st["ctxp"][hi][0:DK, :])

            def n_recip(st, hi):
                st["rrow"][hi] = nrm_pool.tile([1, 1024], f32, tag="rrow",
                                               name="rrow")
                nc.vector.reciprocal_approx_fast(st["rrow"][hi],
                                                 st["srow"][hi])

            def n_bcast(st, hi):
                st["bc"][hi] = nrm_pool.tile([DK, 1024], f32, tag="bc",
                                             name="bc")
                nc.gpsimd.partition_broadcast(st["bc"][hi], st["rrow"][hi])

            def n_mult(st, hi):
                # GpSimd is ~6x slower than DVE for 2-input elementwise
                # (each Q7 core serializes its 16 partitions): DVE only.
                hb = hi * DK
                nc.vector.tensor_tensor(
                    st["ctx_dst"][hb:hb + DK, st["q0"]:st["q0"] + 1024],
                    st["stage"][hi], st["bc"][hi], MULT)

            PHASES = [(0, 0), (0, 1), (1, 0), (1, 1)]
            ctxq = deque()
            pend = None      # normalize state of the previous phase
            for ph, (q2, hp) in enumerate(PHASES):
                q0 = q2 * 1024
                ctxp = [ctx_pool.tile([65, 1024], f32, tag="ctx",
                                      name=f"ctx{hi}") for hi in range(2)]
                # per-iteration extras: {kc: [callable, ...]}
                extras = {}

                def _at(kc_t, fn, _e=extras):
                    kc_t = max(0, min(kc_t, nkc - 1))
                    _e.setdefault(kc_t, []).append(fn)

                if pend is not None:
                    # srow0/drain0 were emitted at the previous phase's end
                    pp = pend
                    _at(0, lambda st=pp: n_srow(st, 1))
                    _at(1, lambda st=pp: n_drain(st, 1, True))
                    _at(1, lambda st=pp: n_recip(st, 0))
                    _at(2, lambda st=pp: n_recip(st, 1))
                    _at(2, lambda st=pp: n_bcast(st, 0))
                    _at(3, lambda st=pp: n_bcast(st, 1))
                    _at(4, lambda st=pp: n_mult(st, 0))
                    _at(5, lambda st=pp: n_mult(st, 1))
                if ph == 2:
                    for lc in range(3):         # lc 0..2 at kc nkc-3..nkc-1
                        _at(nkc - 3 + lc, lambda lc_=lc: emit_oproj_lc(
                            lc_, on_scalar=(lc_ % 2 == 0)))
                if ph == 3:
                    for lc in range(3, 8):      # lc 3..7 at kc 2..6
                        _at(lc - 1, lambda lc_=lc: emit_oproj_lc(
                            lc_, on_scalar=(lc_ % 2 == 0)))

                for kc in range(nkc):
                    if kc == 0 and ph > 0:
                        # phase-boundary HAM heater
                        for _ in range(2):
                            nc.tensor.ldweights(junk[:, 0:P])
                    # delayed ctx: 2 iterations behind scores
                    if ctxq and kc >= 2:
                        ent = ctxq.popleft()
                        emit_ctx(ent, 0)
                        emit_ctx(ent, 1)
                    # scores: each half of each head's [128,1024] is its own
                    # single-bank psum tile with its own exp op, so the
                    # scores->exp->scores WAR loop runs per-half and leaves
                    # ~1us/iter of slack on Sc/DVE to absorb the extras
                    s0h = [s_pool.tile([P, 512], f32, tag=S_TAGS[j], bufs=1,
                                       name=S_TAGS[j]) for j in range(2)]
                    s1h = [s_pool.tile([P, 512], f32, tag=S_TAGS[2 + j],
                                       bufs=1, name=S_TAGS[2 + j])
                           for j in range(2)]
                    for j in range(2):
                        nc.tensor.matmul(
                            s0h[j],
                            lhsT=khp_sb[hp][0][:, kc * P:(kc + 1) * P],
                            rhs=qh_sb[hp][:, q0 + j * 512:q0 + (j + 1) * 512],
                            start=True, stop=True,
                        )
                    for j in range(2):
                        nc.tensor.matmul(
                            s1h[j],
                            lhsT=khp_sb[hp][1][:, kc * P:(kc + 1) * P],
                            rhs=qh_sb[hp][:, q0 + j * 512:q0 + (j + 1) * 512],
                            start=True, stop=True,
                        )
                    for _ in range(NJUNK):
                        nc.tensor.ldweights(junk[:, 0:P])
                    # exp: head0 exact ScalarE, head1 Schraudolph VectorE.
                    # On normalize-carrying phases one h1 exp moves to
                    # ScalarE to rebalance (DVE carries recip+mult there).
                    pt0 = pt_pool.tile([P, 1024], bf16, tag="pt0", name="pt0")
                    for j in range(2):
                        nc.scalar.activation(pt0[:, j * 512:(j + 1) * 512],
                                             s0h[j], EXP)
                    if pend is not None and kc == 6:
                        pt1b = pt_pool.tile([P, 1024], bf16, tag="pt1x",
                                            name="pt1x")
                        for j in range(2):
                            nc.scalar.activation(
                                pt1b[:, j * 512:(j + 1) * 512], s1h[j], EXP)
                        pt1 = pt1b
                    else:
                        pt1i = pt_pool.tile([P, 1024], i16, tag="pt1",
                                            name="pt1")
                        for j in range(2):
                            nc.vector.tensor_scalar(
                                pt1i[:, j * 512:(j + 1) * 512], s1h[j],
                                float(A16), float(B16), MULT, ADD)
                        pt1 = pt1i.bitcast(bf16)
                    ctxq.append((ctxp, hp, (pt0, pt1), kc))
                    for fn in extras.get(kc, ()):
                        fn()
                    if kc == nkc - 1:
                        # phase end: drain the queue completely (the last
                        # exps are just done by the time PE reaches these)
                        while ctxq:
                            ent = ctxq.popleft()
                            emit_ctx(ent, 0)
                            emit_ctx(ent, 1)
                pend = {"ctxp": ctxp, "ctx_dst": ctx_sb[hp], "q0": q0,
                        "srow": {}, "stage": {}, "rrow": {}, "bc": {}}
                # slot-freeing pieces right at the phase end: both engines
                # have an idle window here (the pop-all iteration is PE-long)
                n_srow(pend, 0)
                n_drain(pend, 0, False)

            # ---------------- tail ----------------
            # pre-issue the c2=0 half of oproj lc8-11 (ctx_sb[0] is ready)
            # so the PE stays busy+warm through the final normalize chain
            def oproj_start(lc):
                gi = lc % 2
                if gi == 0:
                    ot_state["t"] = o_pool.tile([P, 2, D], bf16, tag="o",
                                                name="ot")
                ps = s_pool.tile([P, 512], f32, tag=S_TAGS[lc % 4], bufs=1,
                                 name="ps_o")
                nc.tensor.matmul(ps, lhsT=ctx_sb[0][:, lc * P:(lc + 1) * P],
                                 rhs=wo_ap(0), start=True, stop=False)
                return ps, ot_state["t"]

            def oproj_finish(lc, ps, ot, on_scalar):
                nc.tensor.matmul(ps, lhsT=ctx_sb[1][:, lc * P:(lc + 1) * P],
                                 rhs=wo_ap(1), start=False, stop=True)
                gi = lc % 2
                if on_scalar:
                    nc.scalar.copy(ot[:, gi, :], ps)
                else:
                    nc.vector.tensor_copy(ot[:, gi, :], ps)
                if gi == 1:
                    nc.sync.dma_start(
                        o[lc - 1:lc + 1].rearrange("l p d -> p l d"), ot)

            n_recip(pend, 0)
            parts = {}
            for lc in range(8, 12):
                parts[lc] = oproj_start(lc)
            n_srow(pend, 1)
            n_bcast(pend, 0)
            n_drain(pend, 1, True)
            n_recip(pend, 1)
            for _ in range(30):     # HAM heater through the chain wait
                nc.tensor.ldweights(junk[:, 0:P])
            n_mult(pend, 0)
            n_bcast(pend, 1)
            n_mult(pend, 1)
            for lc in range(8, 12):
                ps, ot = parts[lc]
                oproj_finish(lc, ps, ot, on_scalar=(lc % 2 == 0))
            for lc in range(12, 16):
                emit_oproj_lc(lc, on_scalar=(lc % 2 == 0))

    nc.compile()
    return nc


def _get_nc(ndc: int, nkc: int):
    key = ("nc", ndc, nkc)
    if key not in _CACHE:
        _CACHE[key] = _build_nc(ndc, nkc)
    return _CACHE[key]


def _prep_core(core, q, k, v, masks, wq_w, wq_b, wk_w, wk_b, wv_w, wv_b, ndc,
               nkc):
    import ml_dtypes

    bf16 = ml_dtypes.bfloat16
    b, hg = core // 2, core % 2
    rows = slice(hg * GD, (hg + 1) * GD)
    scale = np.float32(1.0 / np.sqrt(DK))
    NKP = nkc * P
    idx = np.nonzero(masks[b])[0]          # unmasked key positions

    def xt_flat(x, compact):
        w = NKP if compact else L
        xt = np.zeros((ndc, P, w), np.float32)
        xs = x[idx] if compact else x      # [nk or L, 512]
        xt[:4, :, :xs.shape[0]] = np.ascontiguousarray(xs.T).reshape(4, P, -1)
        if ndc == 5:
            xt[4, 0, :xs.shape[0]] = 1.0   # ones row for the bias chunk
        # [P, ndc, w] partition-major
        return np.ascontiguousarray(xt.transpose(1, 0, 2)).astype(bf16)

    def w_flat(wT, bias, width):
        w = np.zeros((ndc * P, width), np.float32)
        w[:D] = wT
        if ndc == 5:
            w[D] = bias
        return np.ascontiguousarray(
            w.reshape(ndc, P, width).transpose(1, 0, 2).reshape(P, -1))

    wqT = (wq_w[rows, :].T * scale).astype(np.float32)          # [512, 256]
    wkT = wk_w[rows, :].T.astype(np.float32)
    # v weights: mask/ones column LAST per head (col DK; sums -> ctxp row 64)
    wvT = np.zeros((D, HV), np.float32)
    wvb = np.zeros((HV,), np.float32)
    wvg = wv_w[rows, :]
    for hh in range(HPG):
        wvT[:, hh * 65:hh * 65 + DK] = wvg[hh * DK:(hh + 1) * DK].T
        wvb[hh * 65:hh * 65 + DK] = wv_b[rows][hh * DK:(hh + 1) * DK]
    maskc = np.zeros((NKP,), np.float32)
    maskc[:len(idx)] = 1.0
    qt = xt_flat(q[b], False)              # [P, ndc, L]
    return {
        "qTa": np.ascontiguousarray(qt[:, :, 0:1024].reshape(P, -1)),
        "qTb": np.ascontiguousarray(qt[:, :, 1024:2048].reshape(P, -1)),
        "kT": np.ascontiguousarray(xt_flat(k[b], True).reshape(P, -1)),
        "vT": np.ascontiguousarray(xt_flat(v[b], True).reshape(P, -1)),
        "_wk": w_flat(wkT, wk_b[rows], GD),
        "_wq": w_flat(wqT, wq_b[rows] * scale, GD),
        "_wv": w_flat(wvT, wvb, HV),
        "maskT": np.ascontiguousarray(
            maskc.reshape(nkc, P).T.astype(np.float32)),
    }


def kernel(q, k, v, masks, wq_w, wq_b, wk_w, wk_b, wv_w, wv_b, wo_w, wo_b):
    import ml_dtypes

    from concourse.bass_utils import run_bass_kernel_spmd

    bf16 = ml_dtypes.bfloat16
    q = np.asarray(q, np.float32)
    k = np.asarray(k, np.float32)
    v = np.asarray(v, np.float32)
    masks_np = np.asarray(masks)
    args = [np.asarray(a, np.float32) for a in
            (wq_w, wq_b, wk_w, wk_b, wv_w, wv_b, wo_w, wo_b)]
    wq_w, wq_b, wk_w, wk_b, wv_w, wv_b, wo_w, wo_b = args

    ndc = 5 if (np.any(wq_b) or np.any(wk_b) or np.any(wv_b)) else 4
    # key compaction: pad the max unmasked-key count to a 128 multiple
    max_nk = max(int(np.count_nonzero(masks_np[b])) for b in range(B))
    nkc = max(7, (max_nk + P - 1) // P)
    nc = _get_nc(ndc, nkc)

    in_maps = []
    for core in range(8):
        m = _prep_core(core, q, k, v, masks_np, wq_w, wq_b, wk_w, wk_b,
                       wv_w, wv_b, ndc, nkc)
        hg = core % 2
        rows = slice(hg * GD, (hg + 1) * GD)
        woT = np.ascontiguousarray(
            wo_w[:, rows].T.reshape(2, P, D).transpose(1, 0, 2).reshape(P, -1))
        m["wbT"] = np.concatenate(
            [m.pop("_wk"), m.pop("_wq"), m.pop("_wv"), woT],
            axis=1).astype(bf16)
        in_maps.append(m)

    res = run_bass_kernel_spmd(nc, in_maps, core_ids=list(range(8)),
                               trace=_RUN_OPTS.get("trace", False),
                               tmpdir=_RUN_OPTS.get("tmpdir"))
    _CACHE["last_result"] = res
    outs = res.results

    O = np.zeros((B, L, D), np.float32)
    for b in range(B):
        O[b] = (outs[2 * b]["o"].reshape(L, D).astype(np.float32)
                + outs[2 * b + 1]["o"].reshape(L, D).astype(np.float32))
    O += (wv_b @ wo_w.T + wo_b)[None, None, :] if ndc == 4 else wo_b[None, None, :]
    return O


# revision 27
# speedup vs baseline: 1.1853x; 1.0100x over previous
"""Multi-head attention (B=4, L=2048, D=512, H=8) on 8 Trainium2 NeuronCores.

Sharding: core = (batch b, head-group hg) -> each core handles 1 batch and 4
heads (tensor-parallel column-shard of Wq/Wk/Wv, row-shard of Wo). The two
head-group partial outputs per batch are summed on the host (the TP
all-reduce step of the gather).

v2 engine plan (over the 136us baseline):
  - All DMA'd operands bf16; PE matmuls bf16 with f32 PSUM accumulation.
  - kh stored per-head zero-padded to 128 contraction rows (no PE tiling-mode
    switches; MM time is N-bound so the pad rows are free).
  - Input DMAs merged: kT / weight-blob / qT(2) / vT / mask = 6 issues
    (each DMA_DIRECT2D costs ~0.65us of serial Sync issue time).
  - Deep software pipeline in the attention loop: ctx matmuls run TWO
    iterations behind scores (ctx queue), and each iteration emits ctx
    BEFORE scores so the exp WAR on the single-buffered score PSUM clears
    before the next scores land. PSUM: s0[128,1024] + s1[128,1024] +
    2x ctx[65,1024] = 8 banks.
  - exp split: head0 exact ScalarE ACT (one [128,1024] op), head1 one-shot
    VectorE Schraudolph bf16-bitcast tensor_scalar (renormalization cancels
    the common-mode approximation error).
  - Normalize (deferred into the next phase, one piece per iteration):
      srow:  ScalarE copy ctxp[64:65] -> [1,1024]   (sums row, ones-col trick)
      drain: ScalarE/DVE copy ctxp[0:64] -> stage   (frees ctx psum early)
      recip: DVE reciprocal_approx_fast on srow
      bcast: GpSimd partition_broadcast -> bc[64,1024]
      mult:  head0 GpSimd tensor_tensor (all partition-base-0, aligned),
             head1 DVE tensor_tensor (partition-base shift needs DVE)
    At each phase end the ctx queue is drained completely (the last exps are
    just barely done by the time the PE reaches the popped ctx matmuls), so
    the freeing chain starts immediately at the phase boundary.
  - Output projection: q-half 0 interleaved one l-chunk per iteration late in
    phases (1,0)/(1,1) with all drains on ScalarE; q-half 1 as the tail.
    Output DMA'd bf16 (TP partials summed f32 on host).
  - Projection drains split ScalarE(hp0)/VectorE(hp1) so both engine FIFOs
    stay short ahead of the first exp.
  - Host-side key compaction (masked keys dropped) as in the baseline.
"""
import os
import sys
from collections import deque

import numpy as np

# a wedged NeuronCore (stuck engine state after a killed run) silently
# produces deterministic garbage; resetting cores at runtime init is cheap
os.environ.setdefault("NEURON_RT_RESET_CORES", "1")

for _p in ("/opt/trn_rl_repo", "/root/.axon_site/_ro/trn_rl_repo"):
    if os.path.isdir(_p) and _p not in sys.path:
        sys.path.insert(0, _p)

B, L, D, H = 4, 2048, 512, 8
DK = D // H          # 64
HPG = 4              # heads per group
GD = HPG * DK        # 256
HV = HPG * 65        # v-proj width (per-head mask col + 64 dims)
P = 128
NLB = L // 512       # 4 l-blocks of 512
NLC = L // P         # 16 l chunks

A16 = 128.0 / np.log(2.0)    # Schraudolph bf16 scale
B16 = 16247.9                # zero-mean bias (tuned in simulation)
NJUNK = 1                    # junk LDWEIGHTS per iteration (HAM heater)

_CACHE: dict = {}
_RUN_OPTS: dict = {"trace": False}


def _build_nc(ndc: int, nkc: int):
    """Build + compile the Bass program.

    ndc: 4 normally, 5 when q/k/v biases are nonzero (extra contraction chunk
    carrying a ones row x bias row).
    nkc: number of 128-key chunks after host-side compaction of masked keys.
    """
    from contextlib import ExitStack

    import concourse.bacc as bacc
    import concourse.tile as tile
    from concourse import mybir

    f32 = mybir.dt.float32
    bf16 = mybir.dt.bfloat16
    i16 = mybir.dt.int16
    EXP = mybir.ActivationFunctionType.Exp
    MULT = mybir.AluOpType.mult
    ADD = mybir.AluOpType.add

    nc = bacc.Bacc("TRN2", target_bir_lowering=False, debug=False, num_devices=8)

    NKP = nkc * P
    NKB = (NKP + 511) // 512
    # weight blob layout (free-dim element offsets)
    WK0 = 0
    WQ0 = WK0 + ndc * GD
    WV0 = WQ0 + ndc * GD
    WO0 = WV0 + ndc * HV
    WTOT = WO0 + 2 * D

    kT = nc.dram_tensor("kT", [P, ndc * NKP], bf16, kind="ExternalInput").ap()
    wbT = nc.dram_tensor("wbT", [P, WTOT], bf16, kind="ExternalInput").ap()
    qTa = nc.dram_tensor("qTa", [P, ndc * 1024], bf16, kind="ExternalInput").ap()
    qTb = nc.dram_tensor("qTb", [P, ndc * 1024], bf16, kind="ExternalInput").ap()
    vT = nc.dram_tensor("vT", [P, ndc * NKP], bf16, kind="ExternalInput").ap()
    maskT = nc.dram_tensor("maskT", [P, nkc], f32, kind="ExternalInput").ap()
    o = nc.dram_tensor("o", [NLC, P, D], bf16, kind="ExternalOutput").ap()

    with ExitStack() as ctx:
        tc = ctx.enter_context(tile.TileContext(nc))
        const = ctx.enter_context(tc.tile_pool(name="const", bufs=1))
        persist = ctx.enter_context(tc.tile_pool(name="persist", bufs=1))

        wb_sb = const.tile([P, WTOT], bf16)
        maskp_sb = const.tile([P, nkc], f32)
        dummy_sb = const.tile([1, 8], f32)
        junk = const.tile([P, 512], bf16)
        nc.vector.memset(junk, 0.0)
        # preload the exp table set early (overlaps the projection phase)
        nc.vector.memset(dummy_sb, 0.0)
        nc.scalar.activation(dummy_sb, dummy_sb, EXP)

        def wk_ap(dc):
            return wb_sb[:, WK0 + dc * GD:WK0 + (dc + 1) * GD]

        def wq_ap(dc):
            return wb_sb[:, WQ0 + dc * GD:WQ0 + (dc + 1) * GD]

        def wv_ap(dc):
            return wb_sb[:, WV0 + dc * HV:WV0 + (dc + 1) * HV]

        def wo_ap(c2):
            return wb_sb[:, WO0 + c2 * D:WO0 + (c2 + 1) * D]

        # persistent activations. kh per-head zero-padded to 128 rows.
        qh_sb = [persist.tile([P, L], bf16, name=f"qh{i}") for i in range(2)]
        khp_sb = [[persist.tile([P, NKP], bf16, name=f"khp{i}{j}")
                   for j in range(2)] for i in range(2)]
        # vh col DK(64) = mask/ones column (sums -> ctxp row 64; engine APs
        # must start at 32-aligned partitions, so the ctx rows stay at 0-63)
        vh_sb = persist.tile([P, nkc, HPG, 65], bf16, name="vh")
        ctx_sb = [persist.tile([P, L], bf16, name=f"ctx{i}") for i in range(2)]

        # ---------------- projections ----------------
        with tc.tile_pool(name="xT", bufs=1) as xpool, \
             tc.tile_pool(name="ppsum", bufs=6, space="PSUM") as ppsum:
            # HAM warm-up while the first input DMAs are in flight
            warm = ppsum.tile([P, 512], f32, tag="pp", name="warm")
            for _ in range(24):
                nc.tensor.matmul(warm[:, 0:256], lhsT=junk[:, 0:P],
                                 rhs=junk[:, 0:256], start=True, stop=True)
            for _ in range(20):
                nc.tensor.ldweights(junk[:, 0:P])
            kx = xpool.tile([P, ndc, NKP], bf16, tag="xk", name="kx")
            nc.sync.dma_start(kx, kT.rearrange("p (c w) -> p c w", c=ndc))
            nc.sync.dma_start(wb_sb, wbT)
            qxa = xpool.tile([P, ndc, 1024], bf16, tag="xqa", name="qxa")
            nc.sync.dma_start(qxa, qTa.rearrange("p (c w) -> p c w", c=ndc))
            qxb = xpool.tile([P, ndc, 1024], bf16, tag="xqb", name="qxb")
            nc.sync.dma_start(qxb, qTb.rearrange("p (c w) -> p c w", c=ndc))
            vx = xpool.tile([P, ndc, NKP], bf16, tag="xv", name="vx")
            nc.sync.dma_start(vx, vT.rearrange("p (c w) -> p c w", c=ndc))
            nc.sync.dma_start(maskp_sb, maskT)
            # prewarm the GpSimd custom-op library after input DMAs queued
            # (first partition_broadcast otherwise pays a ~6us IRAM load)
            dummy2_sb = const.tile([1, 8], f32)
            nc.gpsimd.partition_broadcast(dummy2_sb, dummy_sb)
            # khp zero-pad memsets early on DVE
            for hp in range(2):
                for hi in range(2):
                    nc.vector.memset(khp_sb[hp][hi], 0.0)
            # k projection -> khp (per-head zero-padded)
            kps = {}
            for dc in range(ndc):
                for hp in range(2):
                    for lb in range(NKB):
                        nb = min(512, NKP - lb * 512)
                        if dc == 0:
                            kps[hp, lb] = ppsum.tile([P, 512], f32, tag="pp",
                                                     name="ps_k")
                        nc.tensor.matmul(
                            kps[hp, lb][:, 0:nb],
                            lhsT=wk_ap(dc)[:, hp * P:(hp + 1) * P],
                            rhs=kx[:, dc, lb * 512:lb * 512 + nb],
                            start=(dc == 0),
                            stop=(dc == ndc - 1),
                        )
            # drains: hp0 on ScalarE, hp1 on VectorE (parallel FIFOs)
            for (hp, lb), ps in kps.items():
                nb = min(512, NKP - lb * 512)
                for hi in range(2):
                    hb = hi * DK
                    dst = khp_sb[hp][hi][hb:hb + DK, lb * 512:lb * 512 + nb]
                    if hp == 0:
                        nc.scalar.copy(dst, ps[hb:hb + DK, 0:nb])
                    else:
                        nc.vector.tensor_copy(dst, ps[hb:hb + DK, 0:nb])
            # q projection, hp0 first (gates attention start)
            for hp in range(2):
                for lb in range(NLB):
                    qx = qxa if lb < 2 else qxb
                    col = (lb % 2) * 512
                    ps = ppsum.tile([P, 512], f32, tag="pp", name="ps_q")
                    for dc in range(ndc):
                        nc.tensor.matmul(
                            ps,
                            lhsT=wq_ap(dc)[:, hp * P:(hp + 1) * P],
                            rhs=qx[:, dc, col:col + 512],
                            start=(dc == 0),
                            stop=(dc == ndc - 1),
                        )
                    dst = qh_sb[hp][:, lb * 512:(lb + 1) * 512]
                    if hp == 0:
                        nc.scalar.copy(dst, ps)
                    else:
                        nc.vector.tensor_copy(dst, ps)
            # v projection: vh[l, :] with mask fold (keys on partitions)
            for lc in range(nkc):
                ps = ppsum.tile([P, 512], f32, tag="pp", name="ps_v")[:, 0:HV]
                for dc in range(ndc):
                    nc.tensor.matmul(
                        ps,
                        lhsT=vx[:, dc, lc * P:(lc + 1) * P],
                        rhs=wv_ap(dc),
                        start=(dc == 0),
                        stop=(dc == ndc - 1),
                    )
                nc.vector.tensor_scalar_mul(
                    vh_sb[:, lc, :, :], ps.rearrange("p (h d) -> p h d", h=HPG),
                    maskp_sb[:, lc:lc + 1],
                )
                # mask column -> 0/1 (weights there are zero)
                nc.vector.tensor_copy(
                    vh_sb[:, lc, :, DK:DK + 1],
                    maskp_sb[:, lc:lc + 1, None].to_broadcast((P, HPG, 1)),
                )

        # ---------------- attention ----------------
        with tc.tile_pool(name="spsum", bufs=1, space="PSUM") as s_pool, \
             tc.tile_pool(name="cpsum", bufs=2, space="PSUM") as ctx_pool, \
             tc.tile_pool(name="pt", bufs=4) as pt_pool, \
             tc.tile_pool(name="nrm", bufs=4) as nrm_pool, \
             tc.tile_pool(name="osb", bufs=4) as o_pool:

            ot_state = {}
            S_TAGS = ("s0a", "s0b", "s1a", "s1b")

            def emit_oproj_lc(lc, on_scalar=True):
                # one l-chunk of the output projection (borrows s psum)
                gi = lc % 2
                if gi == 0:
                    ot_state["t"] = o_pool.tile([P, 2, D], bf16, tag="o",
                                                name="ot")
                ot = ot_state["t"]
                ps = s_pool.tile([P, 512], f32, tag=S_TAGS[lc % 4], bufs=1,
                                 name="ps_o")
                for c2 in range(2):
                    nc.tensor.matmul(
                        ps,
                        lhsT=ctx_sb[c2][:, lc * P:(lc + 1) * P],
                        rhs=wo_ap(c2),
                        start=(c2 == 0), stop=(c2 == 1),
                    )
                if on_scalar:
                    nc.scalar.copy(ot[:, gi, :], ps)
                else:
                    nc.vector.tensor_copy(ot[:, gi, :], ps)
                if gi == 1:
                    nc.sync.dma_start(
                        o[lc - 1:lc + 1].rearrange("l p d -> p l d"), ot)

            def emit_ctx(ent, hi):
                # ctx accumulation for one delayed iteration, one head
                ctxp_e, hp_e, pts, kcp = ent
                pt = pts[hi]
                vlhsT = vh_sb[:, kcp, 2 * hp_e + hi, :]
                for j in range(2):
                    nc.tensor.matmul(
                        ctxp_e[hi][:, j * 512:(j + 1) * 512],
                        lhsT=vlhsT,
                        rhs=pt[:, j * 512:(j + 1) * 512],
                        start=(kcp == 0), stop=(kcp == nkc - 1),
                    )

            # normalize chain pieces (phase-carried)
            def n_srow(st, hi):
                st["srow"][hi] = nrm_pool.tile([1, 1024], f32, tag="srow",
                                               name="srow")
                nc.scalar.copy(st["srow"][hi], st["ctxp"][hi][64:65, :])

            def n_drain(st, hi, on_scalar):
                st["stage"][hi] = nrm_pool.tile([DK, 1024], f32, tag="stage",
                                                name="stage")
                if on_scalar:
                    nc.scalar.copy(st["stage"][hi], st["ctxp"][hi][0:DK, :])
                else:
                    nc.vector.tensor_copy(st["stage"][hi],
                                          st["ctxp"][hi][0:DK, :])

            def n_recip(st, hi):
                st["rrow"][hi] = nrm_pool.tile([1, 1024], f32, tag="rrow",
                                               name="rrow")
                nc.vector.reciprocal_approx_fast(st["rrow"][hi],
                                                 st["srow"][hi])

            def n_bcast(st, hi):
                st["bc"][hi] = nrm_pool.tile([DK, 1024], f32, tag="bc",
                                             name="bc")
                nc.gpsimd.partition_broadcast(st["bc"][hi], st["rrow"][hi])

            def n_mult(st, hi):
                # GpSimd is ~6x slower than DVE for 2-input elementwise
                # (each Q7 core serializes its 16 partitions): DVE only.
                hb = hi * DK
                nc.vector.tensor_tensor(
                    st["ctx_dst"][hb:hb + DK, st["q0"]:st["q0"] + 1024],
                    st["stage"][hi], st["bc"][hi], MULT)

            PHASES = [(0, 0), (0, 1), (1, 0), (1, 1)]
            ctxq = deque()
            pend = None      # normalize state of the previous phase
            for ph, (q2, hp) in enumerate(PHASES):
                q0 = q2 * 1024
                ctxp = [ctx_pool.tile([65, 1024], f32, tag="ctx",
                                      name=f"ctx{hi}") for hi in range(2)]
                # per-iteration extras: {kc: [callable, ...]}
                extras = {}

                def _at(kc_t, fn, _e=extras):
                    kc_t = max(0, min(kc_t, nkc - 1))
                    _e.setdefault(kc_t, []).append(fn)

                if pend is not None:
                    # srow0/drain0 were emitted at the previous phase's end
                    pp = pend
                    _at(0, lambda st=pp: n_srow(st, 1))
                    _at(2, lambda st=pp: n_drain(st, 1, True))
                    _at(1, lambda st=pp: n_recip(st, 0))
                    _at(2, lambda st=pp: n_recip(st, 1))
                    _at(2, lambda st=pp: n_bcast(st, 0))
                    _at(3, lambda st=pp: n_bcast(st, 1))
                    _at(4, lambda st=pp: n_mult(st, 0))
                    _at(5, lambda st=pp: n_mult(st, 1))
                if ph == 2:
                    for lc in range(3):         # lc 0..2 at kc nkc-3..nkc-1
                        _at(nkc - 3 + lc, lambda lc_=lc: emit_oproj_lc(
                            lc_, on_scalar=(lc_ % 2 == 0)))
                if ph == 3:
                    for lc in range(3, 8):      # lc 3..7 at kc 2..6
                        _at(lc - 1, lambda lc_=lc: emit_oproj_lc(
                            lc_, on_scalar=(lc_ % 2 == 0)))

                for kc in range(nkc):
                    if kc == 0 and ph > 0:
                        # phase-boundary HAM heater
                        for _ in range(2):
                            nc.tensor.ldweights(junk[:, 0:P])
                    # delayed ctx: 2 iterations behind scores
                    if ctxq and kc >= 2:
                        ent = ctxq.popleft()
                        emit_ctx(ent, 0)
                        emit_ctx(ent, 1)
                    # scores: each half of each head's [128,1024] is its own
                    # single-bank psum tile with its own exp op, so the
                    # scores->exp->scores WAR loop runs per-half and leaves
                    # ~1us/iter of slack on Sc/DVE to absorb the extras
                    s0h = [s_pool.tile([P, 512], f32, tag=S_TAGS[j], bufs=1,
                                       name=S_TAGS[j]) for j in range(2)]
                    s1h = [s_pool.tile([P, 512], f32, tag=S_TAGS[2 + j],
                                       bufs=1, name=S_TAGS[2 + j])
                           for j in range(2)]
                    for j in range(2):
                        nc.tensor.matmul(
                            s0h[j],
                            lhsT=khp_sb[hp][0][:, kc * P:(kc + 1) * P],
                            rhs=qh_sb[hp][:, q0 + j * 512:q0 + (j + 1) * 512],
                            start=True, stop=True,
                        )
                    for j in range(2):
                        nc.tensor.matmul(
                            s1h[j],
                            lhsT=khp_sb[hp][1][:, kc * P:(kc + 1) * P],
                            rhs=qh_sb[hp][:, q0 + j * 512:q0 + (j + 1) * 512],
                            start=True, stop=True,
                        )
                    for _ in range(NJUNK):
                        nc.tensor.ldweights(junk[:, 0:P])
                    # exp: head0 exact ScalarE, head1 Schraudolph VectorE.
                    # On normalize-carrying phases one h1 exp moves to
                    # ScalarE to rebalance (DVE carries recip+mult there).
                    pt0 = pt_pool.tile([P, 1024], bf16, tag="pt0", name="pt0")
                    for j in range(2):
                        nc.scalar.activation(pt0[:, j * 512:(j + 1) * 512],
                                             s0h[j], EXP)
                    if pend is not None and kc == 6:
                        pt1b = pt_pool.tile([P, 1024], bf16, tag="pt1x",
                                            name="pt1x")
                        for j in range(2):
                            nc.scalar.activation(
                                pt1b[:, j * 512:(j + 1) * 512], s1h[j], EXP)
                        pt1 = pt1b
                    else:
                        pt1i = pt_pool.tile([P, 1024], i16, tag="pt1",
                                            name="pt1")
                        for j in range(2):
                            nc.vector.tensor_scalar(
                                pt1i[:, j * 512:(j + 1) * 512], s1h[j],
                                float(A16), float(B16), MULT, ADD)
                        pt1 = pt1i.bitcast(bf16)
                    ctxq.append((ctxp, hp, (pt0, pt1), kc))
                    for fn in extras.get(kc, ()):
                        fn()
                    if kc == nkc - 1:
                        # phase end: drain the queue completely (the last
                        # exps are just done by the time PE reaches these)
                        while ctxq:
                            ent = ctxq.popleft()
                            emit_ctx(ent, 0)
                            emit_ctx(ent, 1)
                pend = {"ctxp": ctxp, "ctx_dst": ctx_sb[hp], "q0": q0,
                        "srow": {}, "stage": {}, "rrow": {}, "bc": {}}
                # slot-freeing pieces right at the phase end: both engines
                # have an idle window here (the pop-all iteration is PE-long)
                n_srow(pend, 0)
                n_drain(pend, 0, False)

            # ---------------- tail ----------------
            # pre-issue the c2=0 half of oproj lc8-11 (ctx_sb[0] is ready)
            # so the PE stays busy+warm through the final normalize chain
            def oproj_start(lc):
                gi = lc % 2
                if gi == 0:
                    ot_state["t"] = o_pool.tile([P, 2, D], bf16, tag="o",
                                                name="ot")
                ps = s_pool.tile([P, 512], f32, tag=S_TAGS[lc % 4], bufs=1,
                                 name="ps_o")
                nc.tensor.matmul(ps, lhsT=ctx_sb[0][:, lc * P:(lc + 1) * P],
                                 rhs=wo_ap(0), start=True, stop=False)
                return ps, ot_state["t"]

            def oproj_finish(lc, ps, ot, on_scalar):
                nc.tensor.matmul(ps, lhsT=ctx_sb[1][:, lc * P:(lc + 1) * P],
                                 rhs=wo_ap(1), start=False, stop=True)
                gi = lc % 2
                if on_scalar:
                    nc.scalar.copy(ot[:, gi, :], ps)
                else:
                    nc.vector.tensor_copy(ot[:, gi, :], ps)
                if gi == 1:
                    nc.sync.dma_start(
                        o[lc - 1:lc + 1].rearrange("l p d -> p l d"), ot)

            # final normalize split into q-512 column halves: the a-half
            # (q 1024-1535) unblocks oproj lc8-11 ~3us before the b-half
            rr, bcb = {}, {}

            def recip_h(hi, h2):
                t = nrm_pool.tile([1, 512], f32, tag="rrh", name="rrh")
                nc.vector.reciprocal_approx_fast(
                    t, pend["srow"][hi][:, h2 * 512:(h2 + 1) * 512])
                rr[hi, h2] = t

            def bcast_h(hi, h2):
                t = nrm_pool.tile([DK, 512], f32, tag="bch", name="bch")
                nc.gpsimd.partition_broadcast(t, rr[hi, h2])
                bcb[hi, h2] = t

            def mult_h(hi, h2):
                hb = hi * DK
                c0 = pend["q0"] + h2 * 512
                nc.vector.tensor_tensor(
                    pend["ctx_dst"][hb:hb + DK, c0:c0 + 512],
                    pend["stage"][hi][:, h2 * 512:(h2 + 1) * 512],
                    bcb[hi, h2], MULT)

            parts = {}
            for lc in range(8, 12):
                parts[lc] = oproj_start(lc)
            n_srow(pend, 1)
            recip_h(0, 0)
            n_drain(pend, 1, True)
            recip_h(1, 0)
            bcast_h(0, 0)
            bcast_h(1, 0)
            for _ in range(20):     # HAM heater through the chain wait
                nc.tensor.ldweights(junk[:, 0:P])
            mult_h(0, 0)
            mult_h(1, 0)
            for lc in range(8, 12):
                ps, ot = parts[lc]
                oproj_finish(lc, ps, ot, on_scalar=(lc % 2 == 0))
            recip_h(0, 1)
            recip_h(1, 1)
            bcast_h(0, 1)
            bcast_h(1, 1)
            mult_h(0, 1)
            mult_h(1, 1)
            for lc in range(12, 16):
                emit_oproj_lc(lc, on_scalar=(lc % 2 == 0))

    nc.compile()
    return nc


def _get_nc(ndc: int, nkc: int):
    key = ("nc", ndc, nkc)
    if key not in _CACHE:
        _CACHE[key] = _build_nc(ndc, nkc)
    return _CACHE[key]


def _prep_core(core, q, k, v, masks, wq_w, wq_b, wk_w, wk_b, wv_w, wv_b, ndc,
               nkc):
    import ml_dtypes

    bf16 = ml_dtypes.bfloat16
    b, hg = core // 2, core % 2
    rows = slice(hg * GD, (hg + 1) * GD)
    scale = np.float32(1.0 / np.sqrt(DK))
    NKP = nkc * P
    idx = np.nonzero(masks[b])[0]          # unmasked key positions

    def xt_flat(x, compact):
        w = NKP if compact else L
        xt = np.zeros((ndc, P, w), np.float32)
        xs = x[idx] if compact else x      # [nk or L, 512]
        xt[:4, :, :xs.shape[0]] = np.ascontiguousarray(xs.T).reshape(4, P, -1)
        if ndc == 5:
            xt[4, 0, :xs.shape[0]] = 1.0   # ones row for the bias chunk
        # [P, ndc, w] partition-major
        return np.ascontiguousarray(xt.transpose(1, 0, 2)).astype(bf16)

    def w_flat(wT, bias, width):
        w = np.zeros((ndc * P, width), np.float32)
        w[:D] = wT
        if ndc == 5:
            w[D] = bias
        return np.ascontiguousarray(
            w.reshape(ndc, P, width).transpose(1, 0, 2).reshape(P, -1))

    wqT = (wq_w[rows, :].T * scale).astype(np.float32)          # [512, 256]
    wkT = wk_w[rows, :].T.astype(np.float32)
    # v weights: mask/ones column LAST per head (col DK; sums -> ctxp row 64)
    wvT = np.zeros((D, HV), np.float32)
    wvb = np.zeros((HV,), np.float32)
    wvg = wv_w[rows, :]
    for hh in range(HPG):
        wvT[:, hh * 65:hh * 65 + DK] = wvg[hh * DK:(hh + 1) * DK].T
        wvb[hh * 65:hh * 65 + DK] = wv_b[rows][hh * DK:(hh + 1) * DK]
    maskc = np.zeros((NKP,), np.float32)
    maskc[:len(idx)] = 1.0
    qt = xt_flat(q[b], False)              # [P, ndc, L]
    return {
        "qTa": np.ascontiguousarray(qt[:, :, 0:1024].reshape(P, -1)),
        "qTb": np.ascontiguousarray(qt[:, :, 1024:2048].reshape(P, -1)),
        "kT": np.ascontiguousarray(xt_flat(k[b], True).reshape(P, -1)),
        "vT": np.ascontiguousarray(xt_flat(v[b], True).reshape(P, -1)),
        "_wk": w_flat(wkT, wk_b[rows], GD),
        "_wq": w_flat(wqT, wq_b[rows] * scale, GD),
        "_wv": w_flat(wvT, wvb, HV),
        "maskT": np.ascontiguousarray(
            maskc.reshape(nkc, P).T.astype(np.float32)),
    }


def kernel(q, k, v, masks, wq_w, wq_b, wk_w, wk_b, wv_w, wv_b, wo_w, wo_b):
    import ml_dtypes

    from concourse.bass_utils import run_bass_kernel_spmd

    bf16 = ml_dtypes.bfloat16
    q = np.asarray(q, np.float32)
    k = np.asarray(k, np.float32)
    v = np.asarray(v, np.float32)
    masks_np = np.asarray(masks)
    args = [np.asarray(a, np.float32) for a in
            (wq_w, wq_b, wk_w, wk_b, wv_w, wv_b, wo_w, wo_b)]
    wq_w, wq_b, wk_w, wk_b, wv_w, wv_b, wo_w, wo_b = args

    ndc = 5 if (np.any(wq_b) or np.any(wk_b) or np.any(wv_b)) else 4
    # key compaction: pad the max unmasked-key count to a 128 multiple
    max_nk = max(int(np.count_nonzero(masks_np[b])) for b in range(B))
    nkc = max(7, (max_nk + P - 1) // P)
    nc = _get_nc(ndc, nkc)

    in_maps = []
    for core in range(8):
        m = _prep_core(core, q, k, v, masks_np, wq_w, wq_b, wk_w, wk_b,
                       wv_w, wv_b, ndc, nkc)
        hg = core % 2
        rows = slice(hg * GD, (hg + 1) * GD)
        woT = np.ascontiguousarray(
            wo_w[:, rows].T.reshape(2, P, D).transpose(1, 0, 2).reshape(P, -1))
        m["wbT"] = np.concatenate(
            [m.pop("_wk"), m.pop("_wq"), m.pop("_wv"), woT],
            axis=1).astype(bf16)
        in_maps.append(m)

    res = run_bass_kernel_spmd(nc, in_maps, core_ids=list(range(8)),
                               trace=_RUN_OPTS.get("trace", False),
                               tmpdir=_RUN_OPTS.get("tmpdir"))
    _CACHE["last_result"] = res
    outs = res.results

    O = np.zeros((B, L, D), np.float32)
    for b in range(B):
        O[b] = (outs[2 * b]["o"].reshape(L, D).astype(np.float32)
                + outs[2 * b + 1]["o"].reshape(L, D).astype(np.float32))
    O += (wv_b @ wo_w.T + wo_b)[None, None, :] if ndc == 4 else wo_b[None, None, :]
    return O


# revision 28
# speedup vs baseline: 1.1855x; 1.0002x over previous
"""Multi-head attention (B=4, L=2048, D=512, H=8) on 8 Trainium2 NeuronCores.

Sharding: core = (batch b, head-group hg) -> each core handles 1 batch and 4
heads (tensor-parallel column-shard of Wq/Wk/Wv, row-shard of Wo). The two
head-group partial outputs per batch are summed on the host (the TP
all-reduce step of the gather).

v2 engine plan (over the 136us baseline):
  - All DMA'd operands bf16; PE matmuls bf16 with f32 PSUM accumulation.
  - kh stored per-head zero-padded to 128 contraction rows (no PE tiling-mode
    switches; MM time is N-bound so the pad rows are free).
  - Input DMAs merged: kT / weight-blob / qT(2) / vT / mask = 6 issues
    (each DMA_DIRECT2D costs ~0.65us of serial Sync issue time).
  - Deep software pipeline in the attention loop: ctx matmuls run TWO
    iterations behind scores (ctx queue), and each iteration emits ctx
    BEFORE scores so the exp WAR on the single-buffered score PSUM clears
    before the next scores land. PSUM: s0[128,1024] + s1[128,1024] +
    2x ctx[65,1024] = 8 banks.
  - exp split: head0 exact ScalarE ACT (one [128,1024] op), head1 one-shot
    VectorE Schraudolph bf16-bitcast tensor_scalar (renormalization cancels
    the common-mode approximation error).
  - Normalize (deferred into the next phase, one piece per iteration):
      srow:  ScalarE copy ctxp[64:65] -> [1,1024]   (sums row, ones-col trick)
      drain: ScalarE/DVE copy ctxp[0:64] -> stage   (frees ctx psum early)
      recip: DVE reciprocal_approx_fast on srow
      bcast: GpSimd partition_broadcast -> bc[64,1024]
      mult:  head0 GpSimd tensor_tensor (all partition-base-0, aligned),
             head1 DVE tensor_tensor (partition-base shift needs DVE)
    At each phase end the ctx queue is drained completely (the last exps are
    just barely done by the time the PE reaches the popped ctx matmuls), so
    the freeing chain starts immediately at the phase boundary.
  - Output projection: q-half 0 interleaved one l-chunk per iteration late in
    phases (1,0)/(1,1) with all drains on ScalarE; q-half 1 as the tail.
    Output DMA'd bf16 (TP partials summed f32 on host).
  - Projection drains split ScalarE(hp0)/VectorE(hp1) so both engine FIFOs
    stay short ahead of the first exp.
  - Host-side key compaction (masked keys dropped) as in the baseline.
"""
import os
import sys
from collections import deque

import numpy as np

# a wedged NeuronCore (stuck engine state after a killed run) silently
# produces deterministic garbage; resetting cores at runtime init is cheap
os.environ.setdefault("NEURON_RT_RESET_CORES", "1")

for _p in ("/opt/trn_rl_repo", "/root/.axon_site/_ro/trn_rl_repo"):
    if os.path.isdir(_p) and _p not in sys.path:
        sys.path.insert(0, _p)

B, L, D, H = 4, 2048, 512, 8
DK = D // H          # 64
HPG = 4              # heads per group
GD = HPG * DK        # 256
HV = HPG * 65        # v-proj width (per-head mask col + 64 dims)
P = 128
NLB = L // 512       # 4 l-blocks of 512
NLC = L // P         # 16 l chunks

A16 = 128.0 / np.log(2.0)    # Schraudolph bf16 scale
B16 = 16247.9                # zero-mean bias (tuned in simulation)
NJUNK = 1                    # junk LDWEIGHTS per iteration (HAM heater)

_CACHE: dict = {}
_RUN_OPTS: dict = {"trace": False}


def _build_nc(ndc: int, nkc: int):
    """Build + compile the Bass program.

    ndc: 4 normally, 5 when q/k/v biases are nonzero (extra contraction chunk
    carrying a ones row x bias row).
    nkc: number of 128-key chunks after host-side compaction of masked keys.
    """
    from contextlib import ExitStack

    import concourse.bacc as bacc
    import concourse.tile as tile
    from concourse import mybir

    f32 = mybir.dt.float32
    bf16 = mybir.dt.bfloat16
    i16 = mybir.dt.int16
    EXP = mybir.ActivationFunctionType.Exp
    MULT = mybir.AluOpType.mult
    ADD = mybir.AluOpType.add

    nc = bacc.Bacc("TRN2", target_bir_lowering=False, debug=False, num_devices=8)

    NKP = nkc * P
    NKB = (NKP + 511) // 512
    # weight blob layout (free-dim element offsets)
    WK0 = 0
    WQ0 = WK0 + ndc * GD
    WV0 = WQ0 + ndc * GD
    WO0 = WV0 + ndc * HV
    WTOT = WO0 + 2 * D

    kT = nc.dram_tensor("kT", [P, ndc * NKP], bf16, kind="ExternalInput").ap()
    wbT = nc.dram_tensor("wbT", [P, WTOT], bf16, kind="ExternalInput").ap()
    qTa = nc.dram_tensor("qTa", [P, ndc * 1024], bf16, kind="ExternalInput").ap()
    qTb = nc.dram_tensor("qTb", [P, ndc * 1024], bf16, kind="ExternalInput").ap()
    vT = nc.dram_tensor("vT", [P, ndc * NKP], bf16, kind="ExternalInput").ap()
    maskT = nc.dram_tensor("maskT", [P, nkc], f32, kind="ExternalInput").ap()
    o = nc.dram_tensor("o", [NLC, P, D], bf16, kind="ExternalOutput").ap()

    with ExitStack() as ctx:
        tc = ctx.enter_context(tile.TileContext(nc))
        const = ctx.enter_context(tc.tile_pool(name="const", bufs=1))
        persist = ctx.enter_context(tc.tile_pool(name="persist", bufs=1))

        wb_sb = const.tile([P, WTOT], bf16)
        maskp_sb = const.tile([P, nkc], f32)
        dummy_sb = const.tile([1, 8], f32)
        junk = const.tile([P, 512], bf16)
        nc.vector.memset(junk, 0.0)
        # preload the exp table set early (overlaps the projection phase)
        nc.vector.memset(dummy_sb, 0.0)
        nc.scalar.activation(dummy_sb, dummy_sb, EXP)

        def wk_ap(dc):
            return wb_sb[:, WK0 + dc * GD:WK0 + (dc + 1) * GD]

        def wq_ap(dc):
            return wb_sb[:, WQ0 + dc * GD:WQ0 + (dc + 1) * GD]

        def wv_ap(dc):
            return wb_sb[:, WV0 + dc * HV:WV0 + (dc + 1) * HV]

        def wo_ap(c2):
            return wb_sb[:, WO0 + c2 * D:WO0 + (c2 + 1) * D]

        # persistent activations. kh per-head zero-padded to 128 rows.
        qh_sb = [persist.tile([P, L], bf16, name=f"qh{i}") for i in range(2)]
        khp_sb = [[persist.tile([P, NKP], bf16, name=f"khp{i}{j}")
                   for j in range(2)] for i in range(2)]
        # vh col DK(64) = mask/ones column (sums -> ctxp row 64; engine APs
        # must start at 32-aligned partitions, so the ctx rows stay at 0-63)
        vh_sb = persist.tile([P, nkc, HPG, 65], bf16, name="vh")
        ctx_sb = [persist.tile([P, L], bf16, name=f"ctx{i}") for i in range(2)]

        # ---------------- projections ----------------
        with tc.tile_pool(name="xT", bufs=1) as xpool, \
             tc.tile_pool(name="ppsum", bufs=6, space="PSUM") as ppsum:
            # HAM warm-up while the first input DMAs are in flight
            warm = ppsum.tile([P, 512], f32, tag="pp", name="warm")
            for _ in range(24):
                nc.tensor.matmul(warm[:, 0:256], lhsT=junk[:, 0:P],
                                 rhs=junk[:, 0:256], start=True, stop=True)
            for _ in range(20):
                nc.tensor.ldweights(junk[:, 0:P])
            kx = xpool.tile([P, ndc, NKP], bf16, tag="xk", name="kx")
            nc.sync.dma_start(kx, kT.rearrange("p (c w) -> p c w", c=ndc))
            nc.sync.dma_start(wb_sb, wbT)
            qxa = xpool.tile([P, ndc, 1024], bf16, tag="xqa", name="qxa")
            nc.sync.dma_start(qxa, qTa.rearrange("p (c w) -> p c w", c=ndc))
            qxb = xpool.tile([P, ndc, 1024], bf16, tag="xqb", name="qxb")
            nc.sync.dma_start(qxb, qTb.rearrange("p (c w) -> p c w", c=ndc))
            vx = xpool.tile([P, ndc, NKP], bf16, tag="xv", name="vx")
            nc.sync.dma_start(vx, vT.rearrange("p (c w) -> p c w", c=ndc))
            nc.sync.dma_start(maskp_sb, maskT)
            # prewarm the GpSimd custom-op library after input DMAs queued
            # (first partition_broadcast otherwise pays a ~6us IRAM load)
            dummy2_sb = const.tile([1, 8], f32)
            nc.gpsimd.partition_broadcast(dummy2_sb, dummy_sb)
            # khp zero-pad memsets early on DVE
            for hp in range(2):
                for hi in range(2):
                    nc.vector.memset(khp_sb[hp][hi], 0.0)
            # k projection -> khp (per-head zero-padded)
            kps = {}
            for dc in range(ndc):
                for hp in range(2):
                    for lb in range(NKB):
                        nb = min(512, NKP - lb * 512)
                        if dc == 0:
                            kps[hp, lb] = ppsum.tile([P, 512], f32, tag="pp",
                                                     name="ps_k")
                        nc.tensor.matmul(
                            kps[hp, lb][:, 0:nb],
                            lhsT=wk_ap(dc)[:, hp * P:(hp + 1) * P],
                            rhs=kx[:, dc, lb * 512:lb * 512 + nb],
                            start=(dc == 0),
                            stop=(dc == ndc - 1),
                        )
            # drains: hp0 on ScalarE, hp1 on VectorE (parallel FIFOs)
            for (hp, lb), ps in kps.items():
                nb = min(512, NKP - lb * 512)
                for hi in range(2):
                    hb = hi * DK
                    dst = khp_sb[hp][hi][hb:hb + DK, lb * 512:lb * 512 + nb]
                    if hp == 0:
                        nc.scalar.copy(dst, ps[hb:hb + DK, 0:nb])
                    else:
                        nc.vector.tensor_copy(dst, ps[hb:hb + DK, 0:nb])
            # q projection, hp0 first (gates attention start)
            for hp in range(2):
                for lb in range(NLB):
                    qx = qxa if lb < 2 else qxb
                    col = (lb % 2) * 512
                    ps = ppsum.tile([P, 512], f32, tag="pp", name="ps_q")
                    for dc in range(ndc):
                        nc.tensor.matmul(
                            ps,
                            lhsT=wq_ap(dc)[:, hp * P:(hp + 1) * P],
                            rhs=qx[:, dc, col:col + 512],
                            start=(dc == 0),
                            stop=(dc == ndc - 1),
                        )
                    dst = qh_sb[hp][:, lb * 512:(lb + 1) * 512]
                    if hp == 0:
                        nc.scalar.copy(dst, ps)
                    else:
                        nc.vector.tensor_copy(dst, ps)
            # v projection: vh[l, :] with mask fold (keys on partitions)
            for lc in range(nkc):
                ps = ppsum.tile([P, 512], f32, tag="pp", name="ps_v")[:, 0:HV]
                for dc in range(ndc):
                    nc.tensor.matmul(
                        ps,
                        lhsT=vx[:, dc, lc * P:(lc + 1) * P],
                        rhs=wv_ap(dc),
                        start=(dc == 0),
                        stop=(dc == ndc - 1),
                    )
                nc.vector.tensor_scalar_mul(
                    vh_sb[:, lc, :, :], ps.rearrange("p (h d) -> p h d", h=HPG),
                    maskp_sb[:, lc:lc + 1],
                )
                # mask column -> 0/1 (weights there are zero)
                nc.vector.tensor_copy(
                    vh_sb[:, lc, :, DK:DK + 1],
                    maskp_sb[:, lc:lc + 1, None].to_broadcast((P, HPG, 1)),
                )

        # ---------------- attention ----------------
        with tc.tile_pool(name="spsum", bufs=1, space="PSUM") as s_pool, \
             tc.tile_pool(name="cpsum", bufs=2, space="PSUM") as ctx_pool, \
             tc.tile_pool(name="pt", bufs=4) as pt_pool, \
             tc.tile_pool(name="nrm", bufs=4) as nrm_pool, \
             tc.tile_pool(name="osb", bufs=4) as o_pool:

            ot_state = {}
            S_TAGS = ("s0a", "s0b", "s1a", "s1b")

            def emit_oproj_lc(lc, on_scalar=True):
                # one l-chunk of the output projection (borrows s psum)
                gi = lc % 2
                if gi == 0:
                    ot_state["t"] = o_pool.tile([P, 2, D], bf16, tag="o",
                                                name="ot")
                ot = ot_state["t"]
                ps = s_pool.tile([P, 512], f32, tag=S_TAGS[lc % 4], bufs=1,
                                 name="ps_o")
                for c2 in range(2):
                    nc.tensor.matmul(
                        ps,
                        lhsT=ctx_sb[c2][:, lc * P:(lc + 1) * P],
                        rhs=wo_ap(c2),
                        start=(c2 == 0), stop=(c2 == 1),
                    )
                if on_scalar:
                    nc.scalar.copy(ot[:, gi, :], ps)
                else:
                    nc.vector.tensor_copy(ot[:, gi, :], ps)
                if gi == 1:
                    nc.sync.dma_start(
                        o[lc - 1:lc + 1].rearrange("l p d -> p l d"), ot)

            def emit_ctx(ent, hi):
                # ctx accumulation for one delayed iteration, one head
                ctxp_e, hp_e, pts, kcp = ent
                pt = pts[hi]
                vlhsT = vh_sb[:, kcp, 2 * hp_e + hi, :]
                for j in range(2):
                    nc.tensor.matmul(
                        ctxp_e[hi][:, j * 512:(j + 1) * 512],
                        lhsT=vlhsT,
                        rhs=pt[:, j * 512:(j + 1) * 512],
                        start=(kcp == 0), stop=(kcp == nkc - 1),
                    )

            # normalize chain pieces (phase-carried)
            def n_srow(st, hi):
                st["srow"][hi] = nrm_pool.tile([1, 1024], f32, tag="srow",
                                               name="srow")
                nc.scalar.copy(st["srow"][hi], st["ctxp"][hi][64:65, :])

            def n_drain(st, hi, on_scalar):
                st["stage"][hi] = nrm_pool.tile([DK, 1024], f32, tag="stage",
                                                name="stage")
                if on_scalar:
                    nc.scalar.copy(st["stage"][hi], st["ctxp"][hi][0:DK, :])
                else:
                    nc.vector.tensor_copy(st["stage"][hi],
                                          st["ctxp"][hi][0:DK, :])

            def n_recip(st, hi):
                st["rrow"][hi] = nrm_pool.tile([1, 1024], f32, tag="rrow",
                                               name="rrow")
                nc.vector.reciprocal_approx_fast(st["rrow"][hi],
                                                 st["srow"][hi])

            def n_bcast(st, hi):
                st["bc"][hi] = nrm_pool.tile([DK, 1024], f32, tag="bc",
                                             name="bc")
                nc.gpsimd.partition_broadcast(st["bc"][hi], st["rrow"][hi])

            def n_mult(st, hi):
                # GpSimd is ~6x slower than DVE for 2-input elementwise
                # (each Q7 core serializes its 16 partitions): DVE only.
                hb = hi * DK
                nc.vector.tensor_tensor(
                    st["ctx_dst"][hb:hb + DK, st["q0"]:st["q0"] + 1024],
                    st["stage"][hi], st["bc"][hi], MULT)

            PHASES = [(0, 0), (0, 1), (1, 0), (1, 1)]
            ctxq = deque()
            pend = None      # normalize state of the previous phase
            for ph, (q2, hp) in enumerate(PHASES):
                q0 = q2 * 1024
                ctxp = [ctx_pool.tile([65, 1024], f32, tag="ctx",
                                      name=f"ctx{hi}") for hi in range(2)]
                # per-iteration extras: {kc: [callable, ...]}
                extras = {}

                def _at(kc_t, fn, _e=extras):
                    kc_t = max(0, min(kc_t, nkc - 1))
                    _e.setdefault(kc_t, []).append(fn)

                if pend is not None:
                    # srow0/drain0 were emitted at the previous phase's end
                    pp = pend
                    _at(0, lambda st=pp: n_srow(st, 1))
                    _at(2, lambda st=pp: n_drain(st, 1, True))
                    _at(1, lambda st=pp: n_recip(st, 0))
                    _at(2, lambda st=pp: n_recip(st, 1))
                    _at(2, lambda st=pp: n_bcast(st, 0))
                    _at(3, lambda st=pp: n_bcast(st, 1))
                    _at(4, lambda st=pp: n_mult(st, 0))
                    _at(5, lambda st=pp: n_mult(st, 1))
                if ph == 2:
                    for lc in range(3):         # lc 0..2 at kc nkc-3..nkc-1
                        _at(nkc - 3 + lc, lambda lc_=lc: emit_oproj_lc(
                            lc_, on_scalar=(lc_ % 2 == 0)))
                if ph == 3:
                    for lc in range(3, 8):      # lc 3..7 at kc 2..6
                        _at(lc - 1, lambda lc_=lc: emit_oproj_lc(
                            lc_, on_scalar=(lc_ % 2 == 0)))

                for kc in range(nkc):
                    if kc == 0 and ph > 0:
                        # phase-boundary HAM heater
                        for _ in range(2):
                            nc.tensor.ldweights(junk[:, 0:P])
                    # delayed ctx: 2 iterations behind scores
                    if ctxq and kc >= 2:
                        ent = ctxq.popleft()
                        emit_ctx(ent, 0)
                        emit_ctx(ent, 1)
                    # scores: each half of each head's [128,1024] is its own
                    # single-bank psum tile with its own exp op, so the
                    # scores->exp->scores WAR loop runs per-half and leaves
                    # ~1us/iter of slack on Sc/DVE to absorb the extras
                    s0h = [s_pool.tile([P, 512], f32, tag=S_TAGS[j], bufs=1,
                                       name=S_TAGS[j]) for j in range(2)]
                    s1h = [s_pool.tile([P, 512], f32, tag=S_TAGS[2 + j],
                                       bufs=1, name=S_TAGS[2 + j])
                           for j in range(2)]
                    for j in range(2):
                        nc.tensor.matmul(
                            s0h[j],
                            lhsT=khp_sb[hp][0][:, kc * P:(kc + 1) * P],
                            rhs=qh_sb[hp][:, q0 + j * 512:q0 + (j + 1) * 512],
                            start=True, stop=True,
                        )
                    for j in range(2):
                        nc.tensor.matmul(
                            s1h[j],
                            lhsT=khp_sb[hp][1][:, kc * P:(kc + 1) * P],
                            rhs=qh_sb[hp][:, q0 + j * 512:q0 + (j + 1) * 512],
                            start=True, stop=True,
                        )
                    for _ in range(NJUNK):
                        nc.tensor.ldweights(junk[:, 0:P])
                    # exp: head0 exact ScalarE, head1 Schraudolph VectorE.
                    # On normalize-carrying phases one h1 exp moves to
                    # ScalarE to rebalance (DVE carries recip+mult there).
                    pt0 = pt_pool.tile([P, 1024], bf16, tag="pt0", name="pt0")
                    for j in range(2):
                        nc.scalar.activation(pt0[:, j * 512:(j + 1) * 512],
                                             s0h[j], EXP)
                    if False:   # h1->ScalarE swap: hurts at 2.4GHz (ScalarE
                        pt1b = pt_pool.tile([P, 1024], bf16, tag="pt1x",
                                            name="pt1x")   # is the laggard)
                        for j in range(2):
                            nc.scalar.activation(
                                pt1b[:, j * 512:(j + 1) * 512], s1h[j], EXP)
                        pt1 = pt1b
                    else:
                        pt1i = pt_pool.tile([P, 1024], i16, tag="pt1",
                                            name="pt1")
                        for j in range(2):
                            nc.vector.tensor_scalar(
                                pt1i[:, j * 512:(j + 1) * 512], s1h[j],
                                float(A16), float(B16), MULT, ADD)
                        pt1 = pt1i.bitcast(bf16)
                    ctxq.append((ctxp, hp, (pt0, pt1), kc))
                    for fn in extras.get(kc, ()):
                        fn()
                    if kc == nkc - 1:
                        # phase end: drain the queue completely (the last
                        # exps are just done by the time PE reaches these)
                        while ctxq:
                            ent = ctxq.popleft()
                            emit_ctx(ent, 0)
                            emit_ctx(ent, 1)
                pend = {"ctxp": ctxp, "ctx_dst": ctx_sb[hp], "q0": q0,
                        "srow": {}, "stage": {}, "rrow": {}, "bc": {}}
                # slot-freeing pieces right at the phase end: both engines
                # have an idle window here (the pop-all iteration is PE-long)
                n_srow(pend, 0)
                n_drain(pend, 0, False)

            # ---------------- tail ----------------
            # pre-issue the c2=0 half of oproj lc8-11 (ctx_sb[0] is ready)
            # so the PE stays busy+warm through the final normalize chain
            def oproj_start(lc):
                gi = lc % 2
                if gi == 0:
                    ot_state["t"] = o_pool.tile([P, 2, D], bf16, tag="o",
                                                name="ot")
                ps = s_pool.tile([P, 512], f32, tag=S_TAGS[lc % 4], bufs=1,
                                 name="ps_o")
                nc.tensor.matmul(ps, lhsT=ctx_sb[0][:, lc * P:(lc + 1) * P],
                                 rhs=wo_ap(0), start=True, stop=False)
                return ps, ot_state["t"]

            def oproj_finish(lc, ps, ot, on_scalar):
                nc.tensor.matmul(ps, lhsT=ctx_sb[1][:, lc * P:(lc + 1) * P],
                                 rhs=wo_ap(1), start=False, stop=True)
                gi = lc % 2
                if on_scalar:
                    nc.scalar.copy(ot[:, gi, :], ps)
                else:
                    nc.vector.tensor_copy(ot[:, gi, :], ps)
                if gi == 1:
                    nc.sync.dma_start(
                        o[lc - 1:lc + 1].rearrange("l p d -> p l d"), ot)

            # final normalize split into q-512 column halves: the a-half
            # (q 1024-1535) unblocks oproj lc8-11 ~3us before the b-half
            rr, bcb = {}, {}

            def recip_h(hi, h2):
                t = nrm_pool.tile([1, 512], f32, tag="rrh", name="rrh")
                nc.vector.reciprocal_approx_fast(
                    t, pend["srow"][hi][:, h2 * 512:(h2 + 1) * 512])
                rr[hi, h2] = t

            def bcast_h(hi, h2):
                t = nrm_pool.tile([DK, 512], f32, tag="bch", name="bch")
                nc.gpsimd.partition_broadcast(t, rr[hi, h2])
                bcb[hi, h2] = t

            def mult_h(hi, h2):
                hb = hi * DK
                c0 = pend["q0"] + h2 * 512
                nc.vector.tensor_tensor(
                    pend["ctx_dst"][hb:hb + DK, c0:c0 + 512],
                    pend["stage"][hi][:, h2 * 512:(h2 + 1) * 512],
                    bcb[hi, h2], MULT)

            parts = {}
            for lc in range(8, 12):
                parts[lc] = oproj_start(lc)
            n_srow(pend, 1)
            recip_h(0, 0)
            n_drain(pend, 1, True)
            recip_h(1, 0)
            bcast_h(0, 0)
            bcast_h(1, 0)
            for _ in range(20):     # HAM heater through the chain wait
                nc.tensor.ldweights(junk[:, 0:P])
            mult_h(0, 0)
            mult_h(1, 0)
            for lc in range(8, 12):
                ps, ot = parts[lc]
                oproj_finish(lc, ps, ot, on_scalar=(lc % 2 == 0))
            recip_h(0, 1)
            recip_h(1, 1)
            bcast_h(0, 1)
            bcast_h(1, 1)
            mult_h(0, 1)
            mult_h(1, 1)
            for lc in range(12, 16):
                emit_oproj_lc(lc, on_scalar=(lc % 2 == 0))

    nc.compile()
    return nc


def _get_nc(ndc: int, nkc: int):
    key = ("nc", ndc, nkc)
    if key not in _CACHE:
        _CACHE[key] = _build_nc(ndc, nkc)
    return _CACHE[key]


def _prep_core(core, q, k, v, masks, wq_w, wq_b, wk_w, wk_b, wv_w, wv_b, ndc,
               nkc):
    import ml_dtypes

    bf16 = ml_dtypes.bfloat16
    b, hg = core // 2, core % 2
    rows = slice(hg * GD, (hg + 1) * GD)
    scale = np.float32(1.0 / np.sqrt(DK))
    NKP = nkc * P
    idx = np.nonzero(masks[b])[0]          # unmasked key positions

    def xt_flat(x, compact):
        w = NKP if compact else L
        xt = np.zeros((ndc, P, w), np.float32)
        xs = x[idx] if compact else x      # [nk or L, 512]
        xt[:4, :, :xs.shape[0]] = np.ascontiguousarray(xs.T).reshape(4, P, -1)
        if ndc == 5:
            xt[4, 0, :xs.shape[0]] = 1.0   # ones row for the bias chunk
        # [P, ndc, w] partition-major
        return np.ascontiguousarray(xt.transpose(1, 0, 2)).astype(bf16)

    def w_flat(wT, bias, width):
        w = np.zeros((ndc * P, width), np.float32)
        w[:D] = wT
        if ndc == 5:
            w[D] = bias
        return np.ascontiguousarray(
            w.reshape(ndc, P, width).transpose(1, 0, 2).reshape(P, -1))

    wqT = (wq_w[rows, :].T * scale).astype(np.float32)          # [512, 256]
    wkT = wk_w[rows, :].T.astype(np.float32)
    # v weights: mask/ones column LAST per head (col DK; sums -> ctxp row 64)
    wvT = np.zeros((D, HV), np.float32)
    wvb = np.zeros((HV,), np.float32)
    wvg = wv_w[rows, :]
    for hh in range(HPG):
        wvT[:, hh * 65:hh * 65 + DK] = wvg[hh * DK:(hh + 1) * DK].T
        wvb[hh * 65:hh * 65 + DK] = wv_b[rows][hh * DK:(hh + 1) * DK]
    maskc = np.zeros((NKP,), np.float32)
    maskc[:len(idx)] = 1.0
    qt = xt_flat(q[b], False)              # [P, ndc, L]
    return {
        "qTa": np.ascontiguousarray(qt[:, :, 0:1024].reshape(P, -1)),
        "qTb": np.ascontiguousarray(qt[:, :, 1024:2048].reshape(P, -1)),
        "kT": np.ascontiguousarray(xt_flat(k[b], True).reshape(P, -1)),
        "vT": np.ascontiguousarray(xt_flat(v[b], True).reshape(P, -1)),
        "_wk": w_flat(wkT, wk_b[rows], GD),
        "_wq": w_flat(wqT, wq_b[rows] * scale, GD),
        "_wv": w_flat(wvT, wvb, HV),
        "maskT": np.ascontiguousarray(
            maskc.reshape(nkc, P).T.astype(np.float32)),
    }


def kernel(q, k, v, masks, wq_w, wq_b, wk_w, wk_b, wv_w, wv_b, wo_w, wo_b):
    import ml_dtypes

    from concourse.bass_utils import run_bass_kernel_spmd

    bf16 = ml_dtypes.bfloat16
    q = np.asarray(q, np.float32)
    k = np.asarray(k, np.float32)
    v = np.asarray(v, np.float32)
    masks_np = np.asarray(masks)
    args = [np.asarray(a, np.float32) for a in
            (wq_w, wq_b, wk_w, wk_b, wv_w, wv_b, wo_w, wo_b)]
    wq_w, wq_b, wk_w, wk_b, wv_w, wv_b, wo_w, wo_b = args

    ndc = 5 if (np.any(wq_b) or np.any(wk_b) or np.any(wv_b)) else 4
    # key compaction: pad the max unmasked-key count to a 128 multiple
    max_nk = max(int(np.count_nonzero(masks_np[b])) for b in range(B))
    nkc = max(7, (max_nk + P - 1) // P)
    nc = _get_nc(ndc, nkc)

    in_maps = []
    for core in range(8):
        m = _prep_core(core, q, k, v, masks_np, wq_w, wq_b, wk_w, wk_b,
                       wv_w, wv_b, ndc, nkc)
        hg = core % 2
        rows = slice(hg * GD, (hg + 1) * GD)
        woT = np.ascontiguousarray(
            wo_w[:, rows].T.reshape(2, P, D).transpose(1, 0, 2).reshape(P, -1))
        m["wbT"] = np.concatenate(
            [m.pop("_wk"), m.pop("_wq"), m.pop("_wv"), woT],
            axis=1).astype(bf16)
        in_maps.append(m)

    res = run_bass_kernel_spmd(nc, in_maps, core_ids=list(range(8)),
                               trace=_RUN_OPTS.get("trace", False),
                               tmpdir=_RUN_OPTS.get("tmpdir"))
    _CACHE["last_result"] = res
    outs = res.results

    O = np.zeros((B, L, D), np.float32)
    for b in range(B):
        O[b] = (outs[2 * b]["o"].reshape(L, D).astype(np.float32)
                + outs[2 * b + 1]["o"].reshape(L, D).astype(np.float32))
    O += (wv_b @ wo_w.T + wo_b)[None, None, :] if ndc == 4 else wo_b[None, None, :]
    return O


# revision 29
# speedup vs baseline: 1.1914x; 1.0050x over previous
"""Multi-head attention (B=4, L=2048, D=512, H=8) on 8 Trainium2 NeuronCores.

Sharding: core = (batch b, head-group hg) -> each core handles 1 batch and 4
heads (tensor-parallel column-shard of Wq/Wk/Wv, row-shard of Wo). The two
head-group partial outputs per batch are summed on the host (the TP
all-reduce step of the gather).

v2 engine plan (over the 136us baseline):
  - All DMA'd operands bf16; PE matmuls bf16 with f32 PSUM accumulation.
  - kh stored per-head zero-padded to 128 contraction rows (no PE tiling-mode
    switches; MM time is N-bound so the pad rows are free).
  - Input DMAs merged: kT / weight-blob / qT(2) / vT / mask = 6 issues
    (each DMA_DIRECT2D costs ~0.65us of serial Sync issue time).
  - Deep software pipeline in the attention loop: ctx matmuls run TWO
    iterations behind scores (ctx queue), and each iteration emits ctx
    BEFORE scores so the exp WAR on the single-buffered score PSUM clears
    before the next scores land. PSUM: s0[128,1024] + s1[128,1024] +
    2x ctx[65,1024] = 8 banks.
  - exp split: head0 exact ScalarE ACT (one [128,1024] op), head1 one-shot
    VectorE Schraudolph bf16-bitcast tensor_scalar (renormalization cancels
    the common-mode approximation error).
  - Normalize (deferred into the next phase, one piece per iteration):
      srow:  ScalarE copy ctxp[64:65] -> [1,1024]   (sums row, ones-col trick)
      drain: ScalarE/DVE copy ctxp[0:64] -> stage   (frees ctx psum early)
      recip: DVE reciprocal_approx_fast on srow
      bcast: GpSimd partition_broadcast -> bc[64,1024]
      mult:  head0 GpSimd tensor_tensor (all partition-base-0, aligned),
             head1 DVE tensor_tensor (partition-base shift needs DVE)
    At each phase end the ctx queue is drained completely (the last exps are
    just barely done by the time the PE reaches the popped ctx matmuls), so
    the freeing chain starts immediately at the phase boundary.
  - Output projection: q-half 0 interleaved one l-chunk per iteration late in
    phases (1,0)/(1,1) with all drains on ScalarE; q-half 1 as the tail.
    Output DMA'd bf16 (TP partials summed f32 on host).
  - Projection drains split ScalarE(hp0)/VectorE(hp1) so both engine FIFOs
    stay short ahead of the first exp.
  - Host-side key compaction (masked keys dropped) as in the baseline.
"""
import os
import sys
from collections import deque

import numpy as np

# a wedged NeuronCore (stuck engine state after a killed run) silently
# produces deterministic garbage; resetting cores at runtime init is cheap
os.environ.setdefault("NEURON_RT_RESET_CORES", "1")

for _p in ("/opt/trn_rl_repo", "/root/.axon_site/_ro/trn_rl_repo"):
    if os.path.isdir(_p) and _p not in sys.path:
        sys.path.insert(0, _p)

B, L, D, H = 4, 2048, 512, 8
DK = D // H          # 64
HPG = 4              # heads per group
GD = HPG * DK        # 256
HV = HPG * 65        # v-proj width (per-head mask col + 64 dims)
P = 128
NLB = L // 512       # 4 l-blocks of 512
NLC = L // P         # 16 l chunks

A16 = 128.0 / np.log(2.0)    # Schraudolph bf16 scale
B16 = 16247.9                # zero-mean bias (tuned in simulation)
NJUNK = 1                    # junk LDWEIGHTS per iteration (HAM heater)

_CACHE: dict = {}
_RUN_OPTS: dict = {"trace": False}


def _build_nc(ndc: int, nkc: int):
    """Build + compile the Bass program.

    ndc: 4 normally, 5 when q/k/v biases are nonzero (extra contraction chunk
    carrying a ones row x bias row).
    nkc: number of 128-key chunks after host-side compaction of masked keys.
    """
    from contextlib import ExitStack

    import concourse.bacc as bacc
    import concourse.tile as tile
    from concourse import mybir

    f32 = mybir.dt.float32
    bf16 = mybir.dt.bfloat16
    i16 = mybir.dt.int16
    EXP = mybir.ActivationFunctionType.Exp
    MULT = mybir.AluOpType.mult
    ADD = mybir.AluOpType.add

    nc = bacc.Bacc("TRN2", target_bir_lowering=False, debug=False, num_devices=8)

    NKP = nkc * P
    NKB = (NKP + 511) // 512
    # weight blob layout (free-dim element offsets)
    WK0 = 0
    WQ0 = WK0 + ndc * GD
    WV0 = WQ0 + ndc * GD
    WO0 = WV0 + ndc * HV
    WTOT = WO0 + 2 * D

    kT = nc.dram_tensor("kT", [P, ndc * NKP], bf16, kind="ExternalInput").ap()
    wbT = nc.dram_tensor("wbT", [P, WTOT], bf16, kind="ExternalInput").ap()
    qTa = nc.dram_tensor("qTa", [P, ndc * 1024], bf16, kind="ExternalInput").ap()
    qTb = nc.dram_tensor("qTb", [P, ndc * 1024], bf16, kind="ExternalInput").ap()
    vT = nc.dram_tensor("vT", [P, ndc * NKP], bf16, kind="ExternalInput").ap()
    maskT = nc.dram_tensor("maskT", [P, nkc], f32, kind="ExternalInput").ap()
    o = nc.dram_tensor("o", [NLC, P, D], bf16, kind="ExternalOutput").ap()

    with ExitStack() as ctx:
        tc = ctx.enter_context(tile.TileContext(nc))
        const = ctx.enter_context(tc.tile_pool(name="const", bufs=1))
        persist = ctx.enter_context(tc.tile_pool(name="persist", bufs=1))

        wb_sb = const.tile([P, WTOT], bf16)
        maskp_sb = const.tile([P, nkc], f32)
        dummy_sb = const.tile([1, 8], f32)
        junk = const.tile([P, 512], bf16)
        nc.vector.memset(junk, 0.0)
        # preload the exp table set early (overlaps the projection phase)
        nc.vector.memset(dummy_sb, 0.0)
        nc.scalar.activation(dummy_sb, dummy_sb, EXP)

        def wk_ap(dc):
            return wb_sb[:, WK0 + dc * GD:WK0 + (dc + 1) * GD]

        def wq_ap(dc):
            return wb_sb[:, WQ0 + dc * GD:WQ0 + (dc + 1) * GD]

        def wv_ap(dc):
            return wb_sb[:, WV0 + dc * HV:WV0 + (dc + 1) * HV]

        def wo_ap(c2):
            return wb_sb[:, WO0 + c2 * D:WO0 + (c2 + 1) * D]

        # persistent activations. kh per-head zero-padded to 128 rows.
        qh_sb = [persist.tile([P, L], bf16, name=f"qh{i}") for i in range(2)]
        khp_sb = [[persist.tile([P, NKP], bf16, name=f"khp{i}{j}")
                   for j in range(2)] for i in range(2)]
        # vh col DK(64) = mask/ones column (sums -> ctxp row 64; engine APs
        # must start at 32-aligned partitions, so the ctx rows stay at 0-63)
        vh_sb = persist.tile([P, nkc, HPG, 65], bf16, name="vh")
        ctx_sb = [persist.tile([P, L], bf16, name=f"ctx{i}") for i in range(2)]

        # ---------------- projections ----------------
        with tc.tile_pool(name="xT", bufs=1) as xpool, \
             tc.tile_pool(name="ppsum", bufs=6, space="PSUM") as ppsum:
            # HAM warm-up while the first input DMAs are in flight
            warm = ppsum.tile([P, 512], f32, tag="pp", name="warm")
            for _ in range(24):
                nc.tensor.matmul(warm[:, 0:256], lhsT=junk[:, 0:P],
                                 rhs=junk[:, 0:256], start=True, stop=True)
            for _ in range(20):
                nc.tensor.ldweights(junk[:, 0:P])
            kx = xpool.tile([P, ndc, NKP], bf16, tag="xk", name="kx")
            nc.sync.dma_start(kx, kT.rearrange("p (c w) -> p c w", c=ndc))
            nc.sync.dma_start(wb_sb, wbT)
            qxa = xpool.tile([P, ndc, 1024], bf16, tag="xqa", name="qxa")
            nc.sync.dma_start(qxa, qTa.rearrange("p (c w) -> p c w", c=ndc))
            qxb = xpool.tile([P, ndc, 1024], bf16, tag="xqb", name="qxb")
            nc.sync.dma_start(qxb, qTb.rearrange("p (c w) -> p c w", c=ndc))
            vx = xpool.tile([P, ndc, NKP], bf16, tag="xv", name="vx")
            nc.sync.dma_start(vx, vT.rearrange("p (c w) -> p c w", c=ndc))
            nc.sync.dma_start(maskp_sb, maskT)
            # prewarm the GpSimd custom-op library after input DMAs queued
            # (first partition_broadcast otherwise pays a ~6us IRAM load)
            dummy2_sb = const.tile([1, 8], f32)
            nc.gpsimd.partition_broadcast(dummy2_sb, dummy_sb)
            # khp zero-pad memsets early on DVE
            for hp in range(2):
                for hi in range(2):
                    nc.vector.memset(khp_sb[hp][hi], 0.0)
            # k projection -> khp (per-head zero-padded)
            kps = {}
            for dc in range(ndc):
                for hp in range(2):
                    for lb in range(NKB):
                        nb = min(512, NKP - lb * 512)
                        if dc == 0:
                            kps[hp, lb] = ppsum.tile([P, 512], f32, tag="pp",
                                                     name="ps_k")
                        nc.tensor.matmul(
                            kps[hp, lb][:, 0:nb],
                            lhsT=wk_ap(dc)[:, hp * P:(hp + 1) * P],
                            rhs=kx[:, dc, lb * 512:lb * 512 + nb],
                            start=(dc == 0),
                            stop=(dc == ndc - 1),
                        )
            # drains: hp0 on ScalarE, hp1 on VectorE (parallel FIFOs)
            for (hp, lb), ps in kps.items():
                nb = min(512, NKP - lb * 512)
                for hi in range(2):
                    hb = hi * DK
                    dst = khp_sb[hp][hi][hb:hb + DK, lb * 512:lb * 512 + nb]
                    if hp == 0:
                        nc.scalar.copy(dst, ps[hb:hb + DK, 0:nb])
                    else:
                        nc.vector.tensor_copy(dst, ps[hb:hb + DK, 0:nb])
            # q projection, hp0 first (gates attention start)
            for hp in range(2):
                for lb in range(NLB):
                    qx = qxa if lb < 2 else qxb
                    col = (lb % 2) * 512
                    ps = ppsum.tile([P, 512], f32, tag="pp", name="ps_q")
                    for dc in range(ndc):
                        nc.tensor.matmul(
                            ps,
                            lhsT=wq_ap(dc)[:, hp * P:(hp + 1) * P],
                            rhs=qx[:, dc, col:col + 512],
                            start=(dc == 0),
                            stop=(dc == ndc - 1),
                        )
                    dst = qh_sb[hp][:, lb * 512:(lb + 1) * 512]
                    if hp == 0:
                        nc.scalar.copy(dst, ps)
                    else:
                        nc.vector.tensor_copy(dst, ps)
            # v projection: vh[l, :] with mask fold (keys on partitions)
            for lc in range(nkc):
                ps = ppsum.tile([P, 512], f32, tag="pp", name="ps_v")[:, 0:HV]
                for dc in range(ndc):
                    nc.tensor.matmul(
                        ps,
                        lhsT=vx[:, dc, lc * P:(lc + 1) * P],
                        rhs=wv_ap(dc),
                        start=(dc == 0),
                        stop=(dc == ndc - 1),
                    )
                nc.vector.tensor_scalar_mul(
                    vh_sb[:, lc, :, :], ps.rearrange("p (h d) -> p h d", h=HPG),
                    maskp_sb[:, lc:lc + 1],
                )
            # mask columns -> 0/1 in ONE op for all chunks (weights there are
            # zero, so the muls wrote 0): 9 fewer DVE FIFO slots ahead of the
            # first attention Schraudolph
            nc.vector.tensor_copy(
                vh_sb[:, :, :, DK:DK + 1],
                maskp_sb[:, :, None, None].to_broadcast((P, nkc, HPG, 1)),
            )

        # ---------------- attention ----------------
        with tc.tile_pool(name="spsum", bufs=1, space="PSUM") as s_pool, \
             tc.tile_pool(name="cpsum", bufs=2, space="PSUM") as ctx_pool, \
             tc.tile_pool(name="pt", bufs=4) as pt_pool, \
             tc.tile_pool(name="nrm", bufs=4) as nrm_pool, \
             tc.tile_pool(name="osb", bufs=4) as o_pool:

            ot_state = {}
            S_TAGS = ("s0a", "s0b", "s1a", "s1b")

            def emit_oproj_lc(lc, on_scalar=True):
                # one l-chunk of the output projection (borrows s psum)
                gi = lc % 2
                if gi == 0:
                    ot_state["t"] = o_pool.tile([P, 2, D], bf16, tag="o",
                                                name="ot")
                ot = ot_state["t"]
                ps = s_pool.tile([P, 512], f32, tag=S_TAGS[lc % 4], bufs=1,
                                 name="ps_o")
                for c2 in range(2):
                    nc.tensor.matmul(
                        ps,
                        lhsT=ctx_sb[c2][:, lc * P:(lc + 1) * P],
                        rhs=wo_ap(c2),
                        start=(c2 == 0), stop=(c2 == 1),
                    )
                if on_scalar:
                    nc.scalar.copy(ot[:, gi, :], ps)
                else:
                    nc.vector.tensor_copy(ot[:, gi, :], ps)
                if gi == 1:
                    nc.sync.dma_start(
                        o[lc - 1:lc + 1].rearrange("l p d -> p l d"), ot)

            def emit_ctx(ent, hi):
                # ctx accumulation for one delayed iteration, one head
                ctxp_e, hp_e, pts, kcp = ent
                pt = pts[hi]
                vlhsT = vh_sb[:, kcp, 2 * hp_e + hi, :]
                for j in range(2):
                    nc.tensor.matmul(
                        ctxp_e[hi][:, j * 512:(j + 1) * 512],
                        lhsT=vlhsT,
                        rhs=pt[:, j * 512:(j + 1) * 512],
                        start=(kcp == 0), stop=(kcp == nkc - 1),
                    )

            # normalize chain pieces (phase-carried)
            def n_srow(st, hi):
                st["srow"][hi] = nrm_pool.tile([1, 1024], f32, tag="srow",
                                               name="srow")
                nc.scalar.copy(st["srow"][hi], st["ctxp"][hi][64:65, :])

            def n_drain(st, hi, on_scalar):
                st["stage"][hi] = nrm_pool.tile([DK, 1024], f32, tag="stage",
                                                name="stage")
                if on_scalar:
                    nc.scalar.copy(st["stage"][hi], st["ctxp"][hi][0:DK, :])
                else:
                    nc.vector.tensor_copy(st["stage"][hi],
                                          st["ctxp"][hi][0:DK, :])

            def n_recip(st, hi):
                st["rrow"][hi] = nrm_pool.tile([1, 1024], f32, tag="rrow",
                                               name="rrow")
                nc.vector.reciprocal_approx_fast(st["rrow"][hi],
                                                 st["srow"][hi])

            def n_bcast(st, hi):
                st["bc"][hi] = nrm_pool.tile([DK, 1024], f32, tag="bc",
                                             name="bc")
                nc.gpsimd.partition_broadcast(st["bc"][hi], st["rrow"][hi])

            def n_mult(st, hi):
                # GpSimd is ~6x slower than DVE for 2-input elementwise
                # (each Q7 core serializes its 16 partitions): DVE only.
                hb = hi * DK
                nc.vector.tensor_tensor(
                    st["ctx_dst"][hb:hb + DK, st["q0"]:st["q0"] + 1024],
                    st["stage"][hi], st["bc"][hi], MULT)

            PHASES = [(0, 0), (0, 1), (1, 0), (1, 1)]
            ctxq = deque()
            pend = None      # normalize state of the previous phase
            for ph, (q2, hp) in enumerate(PHASES):
                q0 = q2 * 1024
                ctxp = [ctx_pool.tile([65, 1024], f32, tag="ctx",
                                      name=f"ctx{hi}") for hi in range(2)]
                # per-iteration extras: {kc: [callable, ...]}
                extras = {}

                def _at(kc_t, fn, _e=extras):
                    kc_t = max(0, min(kc_t, nkc - 1))
                    _e.setdefault(kc_t, []).append(fn)

                if pend is not None:
                    # srow0/drain0 were emitted at the previous phase's end
                    pp = pend
                    _at(0, lambda st=pp: n_srow(st, 1))
                    _at(2, lambda st=pp: n_drain(st, 1, True))
                    _at(1, lambda st=pp: n_recip(st, 0))
                    _at(2, lambda st=pp: n_recip(st, 1))
                    _at(2, lambda st=pp: n_bcast(st, 0))
                    _at(3, lambda st=pp: n_bcast(st, 1))
                    _at(4, lambda st=pp: n_mult(st, 0))
                    _at(5, lambda st=pp: n_mult(st, 1))
                if ph == 2:
                    for lc in range(3):         # lc 0..2 at kc nkc-3..nkc-1
                        _at(nkc - 3 + lc, lambda lc_=lc: emit_oproj_lc(
                            lc_, on_scalar=(lc_ % 2 == 0)))
                if ph == 3:
                    for lc in range(3, 8):      # lc 3..7 at kc 2..6
                        _at(lc - 1, lambda lc_=lc: emit_oproj_lc(
                            lc_, on_scalar=(lc_ % 2 == 0)))

                for kc in range(nkc):
                    if kc == 0 and ph > 0:
                        # phase-boundary HAM heater
                        for _ in range(2):
                            nc.tensor.ldweights(junk[:, 0:P])
                    # delayed ctx: 2 iterations behind scores
                    if ctxq and kc >= 2:
                        ent = ctxq.popleft()
                        emit_ctx(ent, 0)
                        emit_ctx(ent, 1)
                    # scores: each half of each head's [128,1024] is its own
                    # single-bank psum tile with its own exp op, so the
                    # scores->exp->scores WAR loop runs per-half and leaves
                    # ~1us/iter of slack on Sc/DVE to absorb the extras
                    s0h = [s_pool.tile([P, 512], f32, tag=S_TAGS[j], bufs=1,
                                       name=S_TAGS[j]) for j in range(2)]
                    s1h = [s_pool.tile([P, 512], f32, tag=S_TAGS[2 + j],
                                       bufs=1, name=S_TAGS[2 + j])
                           for j in range(2)]
                    for j in range(2):
                        nc.tensor.matmul(
                            s0h[j],
                            lhsT=khp_sb[hp][0][:, kc * P:(kc + 1) * P],
                            rhs=qh_sb[hp][:, q0 + j * 512:q0 + (j + 1) * 512],
                            start=True, stop=True,
                        )
                    for j in range(2):
                        nc.tensor.matmul(
                            s1h[j],
                            lhsT=khp_sb[hp][1][:, kc * P:(kc + 1) * P],
                            rhs=qh_sb[hp][:, q0 + j * 512:q0 + (j + 1) * 512],
                            start=True, stop=True,
                        )
                    for _ in range(NJUNK):
                        nc.tensor.ldweights(junk[:, 0:P])
                    # exp: head0 exact ScalarE, head1 Schraudolph VectorE.
                    # On normalize-carrying phases one h1 exp moves to
                    # ScalarE to rebalance (DVE carries recip+mult there).
                    pt0 = pt_pool.tile([P, 1024], bf16, tag="pt0", name="pt0")
                    for j in range(2):
                        nc.scalar.activation(pt0[:, j * 512:(j + 1) * 512],
                                             s0h[j], EXP)
                    if False:   # h1->ScalarE swap: hurts at 2.4GHz (ScalarE
                        pt1b = pt_pool.tile([P, 1024], bf16, tag="pt1x",
                                            name="pt1x")   # is the laggard)
                        for j in range(2):
                            nc.scalar.activation(
                                pt1b[:, j * 512:(j + 1) * 512], s1h[j], EXP)
                        pt1 = pt1b
                    else:
                        pt1i = pt_pool.tile([P, 1024], i16, tag="pt1",
                                            name="pt1")
                        for j in range(2):
                            nc.vector.tensor_scalar(
                                pt1i[:, j * 512:(j + 1) * 512], s1h[j],
                                float(A16), float(B16), MULT, ADD)
                        pt1 = pt1i.bitcast(bf16)
                    ctxq.append((ctxp, hp, (pt0, pt1), kc))
                    for fn in extras.get(kc, ()):
                        fn()
                    if kc == nkc - 1:
                        # phase end: drain the queue completely (the last
                        # exps are just done by the time PE reaches these)
                        while ctxq:
                            ent = ctxq.popleft()
                            emit_ctx(ent, 0)
                            emit_ctx(ent, 1)
                pend = {"ctxp": ctxp, "ctx_dst": ctx_sb[hp], "q0": q0,
                        "srow": {}, "stage": {}, "rrow": {}, "bc": {}}
                # slot-freeing pieces right at the phase end: both engines
                # have an idle window here (the pop-all iteration is PE-long)
                n_srow(pend, 0)
                n_drain(pend, 0, False)

            # ---------------- tail ----------------
            # pre-issue the c2=0 half of oproj lc8-11 (ctx_sb[0] is ready)
            # so the PE stays busy+warm through the final normalize chain
            def oproj_start(lc):
                gi = lc % 2
                if gi == 0:
                    ot_state["t"] = o_pool.tile([P, 2, D], bf16, tag="o",
                                                name="ot")
                ps = s_pool.tile([P, 512], f32, tag=S_TAGS[lc % 4], bufs=1,
                                 name="ps_o")
                nc.tensor.matmul(ps, lhsT=ctx_sb[0][:, lc * P:(lc + 1) * P],
                                 rhs=wo_ap(0), start=True, stop=False)
                return ps, ot_state["t"]

            def oproj_finish(lc, ps, ot, on_scalar):
                nc.tensor.matmul(ps, lhsT=ctx_sb[1][:, lc * P:(lc + 1) * P],
                                 rhs=wo_ap(1), start=False, stop=True)
                gi = lc % 2
                if on_scalar:
                    nc.scalar.copy(ot[:, gi, :], ps)
                else:
                    nc.vector.tensor_copy(ot[:, gi, :], ps)
                if gi == 1:
                    nc.sync.dma_start(
                        o[lc - 1:lc + 1].rearrange("l p d -> p l d"), ot)

            # final normalize split into q-512 column halves: the a-half
            # (q 1024-1535) unblocks oproj lc8-11 ~3us before the b-half
            rr, bcb = {}, {}

            def recip_h(hi, h2):
                t = nrm_pool.tile([1, 512], f32, tag="rrh", name="rrh")
                nc.vector.reciprocal_approx_fast(
                    t, pend["srow"][hi][:, h2 * 512:(h2 + 1) * 512])
                rr[hi, h2] = t

            def bcast_h(hi, h2):
                t = nrm_pool.tile([DK, 512], f32, tag="bch", name="bch")
                nc.gpsimd.partition_broadcast(t, rr[hi, h2])
                bcb[hi, h2] = t

            def mult_h(hi, h2):
                hb = hi * DK
                c0 = pend["q0"] + h2 * 512
                nc.vector.tensor_tensor(
                    pend["ctx_dst"][hb:hb + DK, c0:c0 + 512],
                    pend["stage"][hi][:, h2 * 512:(h2 + 1) * 512],
                    bcb[hi, h2], MULT)

            parts = {}
            for lc in range(8, 12):
                parts[lc] = oproj_start(lc)
            n_srow(pend, 1)
            recip_h(0, 0)
            n_drain(pend, 1, True)
            recip_h(1, 0)
            bcast_h(0, 0)
            bcast_h(1, 0)
            for _ in range(20):     # HAM heater through the chain wait
                nc.tensor.ldweights(junk[:, 0:P])
            mult_h(0, 0)
            mult_h(1, 0)
            for lc in range(8, 12):
                ps, ot = parts[lc]
                oproj_finish(lc, ps, ot, on_scalar=(lc % 2 == 0))
            recip_h(0, 1)
            recip_h(1, 1)
            bcast_h(0, 1)
            bcast_h(1, 1)
            mult_h(0, 1)
            mult_h(1, 1)
            for lc in range(12, 16):
                emit_oproj_lc(lc, on_scalar=(lc % 2 == 0))

    nc.compile()
    return nc


def _get_nc(ndc: int, nkc: int):
    key = ("nc", ndc, nkc)
    if key not in _CACHE:
        _CACHE[key] = _build_nc(ndc, nkc)
    return _CACHE[key]


def _prep_core(core, q, k, v, masks, wq_w, wq_b, wk_w, wk_b, wv_w, wv_b, ndc,
               nkc):
    import ml_dtypes

    bf16 = ml_dtypes.bfloat16
    b, hg = core // 2, core % 2
    rows = slice(hg * GD, (hg + 1) * GD)
    scale = np.float32(1.0 / np.sqrt(DK))
    NKP = nkc * P
    idx = np.nonzero(masks[b])[0]          # unmasked key positions

    def xt_flat(x, compact):
        w = NKP if compact else L
        xt = np.zeros((ndc, P, w), np.float32)
        xs = x[idx] if compact else x      # [nk or L, 512]
        xt[:4, :, :xs.shape[0]] = np.ascontiguousarray(xs.T).reshape(4, P, -1)
        if ndc == 5:
            xt[4, 0, :xs.shape[0]] = 1.0   # ones row for the bias chunk
        # [P, ndc, w] partition-major
        return np.ascontiguousarray(xt.transpose(1, 0, 2)).astype(bf16)

    def w_flat(wT, bias, width):
        w = np.zeros((ndc * P, width), np.float32)
        w[:D] = wT
        if ndc == 5:
            w[D] = bias
        return np.ascontiguousarray(
            w.reshape(ndc, P, width).transpose(1, 0, 2).reshape(P, -1))

    wqT = (wq_w[rows, :].T * scale).astype(np.float32)          # [512, 256]
    wkT = wk_w[rows, :].T.astype(np.float32)
    # v weights: mask/ones column LAST per head (col DK; sums -> ctxp row 64)
    wvT = np.zeros((D, HV), np.float32)
    wvb = np.zeros((HV,), np.float32)
    wvg = wv_w[rows, :]
    for hh in range(HPG):
        wvT[:, hh * 65:hh * 65 + DK] = wvg[hh * DK:(hh + 1) * DK].T
        wvb[hh * 65:hh * 65 + DK] = wv_b[rows][hh * DK:(hh + 1) * DK]
    maskc = np.zeros((NKP,), np.float32)
    maskc[:len(idx)] = 1.0
    qt = xt_flat(q[b], False)              # [P, ndc, L]
    return {
        "qTa": np.ascontiguousarray(qt[:, :, 0:1024].reshape(P, -1)),
        "qTb": np.ascontiguousarray(qt[:, :, 1024:2048].reshape(P, -1)),
        "kT": np.ascontiguousarray(xt_flat(k[b], True).reshape(P, -1)),
        "vT": np.ascontiguousarray(xt_flat(v[b], True).reshape(P, -1)),
        "_wk": w_flat(wkT, wk_b[rows], GD),
        "_wq": w_flat(wqT, wq_b[rows] * scale, GD),
        "_wv": w_flat(wvT, wvb, HV),
        "maskT": np.ascontiguousarray(
            maskc.reshape(nkc, P).T.astype(np.float32)),
    }


def kernel(q, k, v, masks, wq_w, wq_b, wk_w, wk_b, wv_w, wv_b, wo_w, wo_b):
    import ml_dtypes

    from concourse.bass_utils import run_bass_kernel_spmd

    bf16 = ml_dtypes.bfloat16
    q = np.asarray(q, np.float32)
    k = np.asarray(k, np.float32)
    v = np.asarray(v, np.float32)
    masks_np = np.asarray(masks)
    args = [np.asarray(a, np.float32) for a in
            (wq_w, wq_b, wk_w, wk_b, wv_w, wv_b, wo_w, wo_b)]
    wq_w, wq_b, wk_w, wk_b, wv_w, wv_b, wo_w, wo_b = args

    ndc = 5 if (np.any(wq_b) or np.any(wk_b) or np.any(wv_b)) else 4
    # key compaction: pad the max unmasked-key count to a 128 multiple
    max_nk = max(int(np.count_nonzero(masks_np[b])) for b in range(B))
    nkc = max(7, (max_nk + P - 1) // P)
    nc = _get_nc(ndc, nkc)

    in_maps = []
    for core in range(8):
        m = _prep_core(core, q, k, v, masks_np, wq_w, wq_b, wk_w, wk_b,
                       wv_w, wv_b, ndc, nkc)
        hg = core % 2
        rows = slice(hg * GD, (hg + 1) * GD)
        woT = np.ascontiguousarray(
            wo_w[:, rows].T.reshape(2, P, D).transpose(1, 0, 2).reshape(P, -1))
        m["wbT"] = np.concatenate(
            [m.pop("_wk"), m.pop("_wq"), m.pop("_wv"), woT],
            axis=1).astype(bf16)
        in_maps.append(m)

    res = run_bass_kernel_spmd(nc, in_maps, core_ids=list(range(8)),
                               trace=_RUN_OPTS.get("trace", False),
                               tmpdir=_RUN_OPTS.get("tmpdir"))
    _CACHE["last_result"] = res
    outs = res.results

    O = np.zeros((B, L, D), np.float32)
    for b in range(B):
        O[b] = (outs[2 * b]["o"].reshape(L, D).astype(np.float32)
                + outs[2 * b + 1]["o"].reshape(L, D).astype(np.float32))
    O += (wv_b @ wo_w.T + wo_b)[None, None, :] if ndc == 4 else wo_b[None, None, :]
    return O
